# revision 3
# baseline (speedup 1.0000x reference)
"""Trainium2 Bass kernel for a GPT-style transformer block.

Reference computation (B=4, T=2048, d=1024, 16 heads, dff=4096, fp32):
    h  = LN1(x);  qkv = h @ w_attn + b_attn
    y  = causal_attention(q, k, v);  x1 = x + y @ w_proj + b_proj
    h2 = LN2(x1); out = x1 + gelu(h2 @ w_fc + b_fc) @ w_fc2 + b_fc2

Sharding over 8 NeuronCores (one trn2 chip), STRIPED token ownership:
  core c owns token strip [2048*b + 256*c, +256) of every batch b (1024
  tokens total).  This alignment makes each LN1 AllGather chunk ci
  deliver exactly batch ci's tokens, and lets the per-head attention
  output redistribute with one small AllToAll per batch, pipelined
  behind the next batch's attention.

  - ph1: LN1 over own tokens in 4 chunks of 256; each chunk's h^T shard
    (bf16) AllGathers as soon as it is ready.  A 1-byte dummy AllGather
    at kernel start absorbs the ~50us collective-init barrier.
  - ph2: per gathered chunk, q^T/k^T/v^T (bf16, both heads stacked
    on 128 partitions) for this core's 2 heads over that batch.
  - ph3: causal attention, batch-major.  Scores for the two heads run
    CONCURRENTLY as row-tiled K=64 matmuls (rows 0-63 / 64-127 of the
    PE array) into separate PSUM banks.  exp on ACT; causal masking by
    a 0/1 bf16 multiply on DVE (diagonal blocks only); softmax
    denominator comes from 64 ones-columns appended to v, and the
    normalization reciprocal is computed as exp(-ln(s)) on ACT (DVE
    reciprocal is ~16x slower).  After each batch, one AllToAll
    redistributes y to token owners, overlapped with the next batch.
  - ph4/ph5: token-parallel proj+residual+LN2 and MLP with full-width
    weights, as in the reference.

Matmul-shape notes: matmul time = moving-free-dim cycles (independent
of K and M), so the 64-ones columns and K=64 padding are free; what
matters is slot count, which row-tiling halves for scores.  Bias
matmuls (ones-row trick) are skipped at build time when the bias
vectors are all zero (they are, for this problem's inputs).
"""

import sys

import numpy as np
import ml_dtypes

sys.path.insert(0, "/opt/trn_rl_repo")

import concourse.bass as bass  # noqa: E402
import concourse.mybir as mybir  # noqa: E402
import concourse.tile as tile  # noqa: E402
from concourse import bacc  # noqa: E402
from concourse.bass_utils import run_bass_kernel_spmd  # noqa: E402
from concourse.masks import make_identity  # noqa: E402

B, T, D, H, HD, DFF = 4, 2048, 1024, 16, 64, 4096
EPS = 1e-5
NCORES = 8
TOK = B * T            # 8192 flattened tokens
TOWN = TOK // NCORES   # 1024 tokens owned per core
STR = 256              # per-batch strip owned per core
P = 128
F32 = mybir.dt.float32
BF16 = mybir.dt.bfloat16
Act = mybir.ActivationFunctionType
Alu = mybir.AluOpType
AX = mybir.AxisListType
BF = ml_dtypes.bfloat16


def build(use_bproj=True, use_bfc2=True):
    nc = bacc.Bacc("TRN2", target_bir_lowering=False, debug=False, num_devices=NCORES)

    def inp(name, shape, dt=F32):
        return nc.dram_tensor(name, shape, dt, kind="ExternalInput").ap()

    x_own = inp("x_own", [TOWN, D])
    wq = inp("wq", [D, P], BF16)
    wk = inp("wk", [D, P], BF16)
    wv = inp("wv", [D, P], BF16)
    bq = inp("bq", [P, 1])
    bk = inp("bk", [P, 1])
    bv = inp("bv", [P, 1])
    ln1w = inp("ln1w", [P, 8])
    ln1b = inp("ln1b", [P, 8])
    ln2w = inp("ln2w", [P, 8])
    ln2b = inp("ln2b", [P, 8])
    wproj = inp("wproj", [D, D], BF16)
    bproj = inp("bproj", [1, D], BF16)
    wfc = inp("wfc", [D, DFF], BF16)
    bfc = inp("bfc", [P, DFF // P])
    wfc2 = inp("wfc2", [DFF, D], BF16)
    bfc2 = inp("bfc2", [1, D], BF16)
    out_own = nc.dram_tensor("out", [TOWN, D], F32, kind="ExternalOutput").ap()

    groups = [list(range(NCORES))]

    with tile.TileContext(nc) as tc:
        with (
            tc.tile_pool(name="const", bufs=1) as cst,
            tc.tile_pool(name="dram", bufs=1, space="DRAM") as dram,
        ):
            # warm up the collective stream immediately: the first cc op
            # pays a ~50us init/rendezvous cost; make it a 1-byte dummy
            # that overlaps ph1 compute.
            warm_in = dram.tile([1, 1], mybir.dt.uint8, name="warm_in")
            warm_out = dram.tile([NCORES, 1], mybir.dt.uint8, addr_space="Shared",
                                 name="warm_out")
            nc.gpsimd.collective_compute(
                "AllGather", Alu.bypass, replica_groups=groups,
                ins=[warm_in[:]], outs=[warm_out[:]],
            )

            # ---------------- constants ----------------
            ident = cst.tile([P, P], F32)
            make_identity(nc, ident)
            ident_bf = cst.tile([P, P], BF16)
            make_identity(nc, ident_bf)
            ones_b = None
            if use_bproj or use_bfc2:
                ones_f = cst.tile([1, P], F32)
                nc.vector.memset(ones_f[:], 1.0)
                ones_b = cst.tile([1, P], BF16)
                nc.scalar.copy(ones_b[:], ones_f[:])
            ones_half = cst.tile([P, 16, HD], BF16)
            nc.vector.memset(ones_half[:], 1.0)
            ln1w_sb = cst.tile([P, 8], F32)
            nc.sync.dma_start(ln1w_sb[:], ln1w)
            ln1b_sb = cst.tile([P, 8], F32)
            nc.sync.dma_start(ln1b_sb[:], ln1b)
            ln2w_sb = cst.tile([P, 8], F32)
            nc.sync.dma_start(ln2w_sb[:], ln2w)
            ln2b_sb = cst.tile([P, 8], F32)
            nc.sync.dma_start(ln2b_sb[:], ln2b)
            bq_sb = cst.tile([P, 1], F32)
            nc.sync.dma_start(bq_sb[:], bq)
            bk_sb = cst.tile([P, 1], F32)
            nc.sync.dma_start(bk_sb[:], bk)
            bv_sb = cst.tile([P, 1], F32)
            nc.sync.dma_start(bv_sb[:], bv)
            bproj_sb = None
            if use_bproj:
                bproj_sb = cst.tile([1, D], BF16)
                nc.sync.dma_start(bproj_sb[:], bproj)
            bfc_sb = cst.tile([P, DFF // P], F32)
            nc.sync.dma_start(bfc_sb[:], bfc)
            bfc2_sb = None
            if use_bfc2:
                bfc2_sb = cst.tile([1, D], BF16)
                nc.sync.dma_start(bfc2_sb[:], bfc2)
            # 0/1 causal masks for the 4 diagonal offsets, replicated for
            # both heads: mask01[s][i, h, j] = 1 if i <= j - 128*s else 0
            mask01 = cst.tile([P, 4, 2, 512], BF16)
            nc.vector.memset(mask01[:], 1.0)
            for s in range(4):
                for h in range(2):
                    nc.gpsimd.affine_select(
                        out=mask01[:, s, h, :],
                        in_=mask01[:, s, h, :],
                        pattern=[[1, 512]],
                        channel_multiplier=-1,
                        base=-128 * s,
                        compare_op=Alu.is_ge,
                        fill=0.0,
                    )

            # DRAM intermediates.
            # hT chunk ci: own tokens [256ci, 256ci+256) -> gathered chunk
            # ci holds batch ci tokens as [8 strips, ...].
            NCH = 4
            hT_dram = [dram.tile([D, STR], BF16, name=f"hq{i}") for i in range(NCH)]
            hT_full = [dram.tile([NCORES * D, STR], BF16, addr_space="Shared",
                                 name=f"hfq{i}") for i in range(NCH)]
            # per-batch y AllToAll: slice r = my 2 heads' y for core r's
            # strip of this batch.
            y_send = [dram.tile([NCORES, 2, HD, STR], BF16, name=f"ys{b}")
                      for b in range(B)]
            y_recv = [dram.tile([NCORES, 2, HD, STR], BF16, name=f"yr{b}")
                      for b in range(B)]

            # =========================================================
            # Phase 1: LN1 over own tokens, 4 chunks -> AllGather each
            # =========================================================
            def layernorm_tile(pool, xt, w_sb, b_sb, ps_pool, dstT, t):
                """LN a [128, D] token tile and write transposed blocks
                (with gamma/beta applied) into dstT[:, dblk, t, :] (bf16)."""
                ssum = pool.tile([P, 1], F32, tag="ssum")
                nc.vector.reduce_sum(ssum[:], xt[:], axis=AX.X)
                mean = pool.tile([P, 1], F32, tag="mean")
                nc.scalar.mul(mean[:], ssum[:], 1.0 / D)
                sq = pool.tile([P, D], F32, tag="sq")
                sumsq = pool.tile([P, 1], F32, tag="sumsq")
                nc.scalar.activation(sq[:], xt[:], Act.Square, accum_out=sumsq[:])
                msq = pool.tile([P, 1], F32, tag="msq")
                nc.vector.tensor_tensor(msq[:], mean[:], mean[:], op=Alu.mult)
                var = pool.tile([P, 1], F32, tag="var")
                nc.vector.tensor_scalar(var[:], sumsq[:], 1.0 / D, EPS, Alu.mult, Alu.add)
                nc.vector.tensor_tensor(var[:], var[:], msq[:], op=Alu.subtract)
                rinv = pool.tile([P, 1], F32, tag="rinv")
                nc.vector.reciprocal(rinv[:], var[:])
                rstd = pool.tile([P, 1], F32, tag="rstd")
                nc.scalar.sqrt(rstd[:], rinv[:])
                hh = pool.tile([P, D], F32, tag="hh")
                nc.vector.tensor_scalar(
                    hh[:], xt[:], mean[:], rstd[:], Alu.subtract, Alu.mult
                )
                for dblk in range(8):
                    pt = ps_pool.tile([P, P], F32, tag="lnt")
                    nc.tensor.transpose(pt[:], hh[:, dblk * P : (dblk + 1) * P], ident[:])
                    nc.scalar.activation(
                        dstT[:, dblk, t, :],
                        pt[:],
                        Act.Identity,
                        bias=b_sb[:, dblk : dblk + 1],
                        scale=w_sb[:, dblk : dblk + 1],
                    )

            with (
                tc.tile_pool(name="ph1", bufs=2) as ph1,
                tc.tile_pool(name="ph1T", bufs=1) as ph1T,
                tc.tile_pool(name="psA", bufs=2, space="PSUM") as psA,
                nc.named_scope("ph1_ln1"),
            ):
                hT_asm = ph1T.tile([P, 8, 8, P], BF16)  # [p, dblk, t, j]
                for ci in range(NCH):
                    for t in range(2 * ci, 2 * ci + 2):
                        xt = ph1.tile([P, D], F32, tag="xt")
                        nc.sync.dma_start(xt[:], x_own[t * P : (t + 1) * P, :])
                        layernorm_tile(ph1, xt, ln1w_sb, ln1b_sb, psA, hT_asm, t)
                    hTv = hT_dram[ci].rearrange("(dblk p) t -> p dblk t", p=P)
                    for dblk in range(8):
                        nc.sync.dma_start(
                            hTv[:, dblk, :],
                            hT_asm[:, dblk, 2 * ci : 2 * ci + 2, :],
                        )
                    nc.gpsimd.collective_compute(
                        "AllGather", Alu.bypass, replica_groups=groups,
                        ins=[hT_dram[ci][:]], outs=[hT_full[ci][:]],
                    )

            # =========================================================
            # Phase 2: q^T, k^T, v^T (bf16, both heads stacked) per
            # gathered chunk.  Gathered chunk ci = batch ci, strips r.
            # token col in q/k/vT: tile16 = 4*ci + r//2, off = (r%2)*256
            # =========================================================
            with tc.tile_pool(name="qkv", bufs=1) as qkvp:
                qT = qkvp.tile([P, 16, 512], BF16)
                kT = qkvp.tile([P, 16, 512], BF16)
                vT = qkvp.tile([P, 16, 512], BF16)
                with (
                    tc.tile_pool(name="wqkv", bufs=1) as wp,
                    tc.tile_pool(name="ph2", bufs=3) as ph2,
                    tc.tile_pool(name="psB", bufs=3, space="PSUM") as psB,
                    nc.named_scope("ph2_qkv"),
                ):
                    wq_sb = wp.tile([P, 8, P], BF16)
                    nc.sync.dma_start(wq_sb[:], wq.rearrange("(ko p) m -> p ko m", p=P))
                    wk_sb = wp.tile([P, 8, P], BF16)
                    nc.sync.dma_start(wk_sb[:], wk.rearrange("(ko p) m -> p ko m", p=P))
                    wv_sb = wp.tile([P, 8, P], BF16)
                    nc.sync.dma_start(wv_sb[:], wv.rearrange("(ko p) m -> p ko m", p=P))
                    hfvs = [hq.rearrange("(r ko p) t -> r p ko t", p=P, ko=8)
                            for hq in hT_full]
                    for ci in range(NCH):
                        hfv = hfvs[ci]
                        for rr in range(8):
                            ht = ph2.tile([P, 8, STR], BF16, tag="ht")
                            nc.sync.dma_start(ht[:], hfv[rr])
                            sl = (slice(None), 4 * ci + rr // 2,
                                  slice((rr % 2) * STR, (rr % 2) * STR + STR))
                            for wi, (w_sb, b_sb, dstT) in enumerate(
                                ((wq_sb, bq_sb, qT), (wk_sb, bk_sb, kT),
                                 (wv_sb, bv_sb, vT))
                            ):
                                ps = psB.tile([P, STR], F32, tag="qkvps")
                                for ko in range(8):
                                    nc.tensor.matmul(
                                        ps[:], w_sb[:, ko, :], ht[:, ko, :],
                                        start=(ko == 0), stop=(ko == 7),
                                    )
                                nc.scalar.activation(
                                    dstT[sl], ps[:], Act.Identity, bias=b_sb[:],
                                )

                # =====================================================
                # Phase 3: causal attention, batch-major; both heads
                # concurrently via row-tiled K=64 score matmuls.
                # =====================================================
                with (
                    tc.tile_pool(name="ph3", bufs=4) as ph3,
                    tc.tile_pool(name="ph3s", bufs=3) as ph3s,
                    tc.tile_pool(name="vnp", bufs=2) as vnp,
                    tc.tile_pool(name="psQK", bufs=2, space="PSUM") as psQK,
                    tc.tile_pool(name="psY", bufs=1, space="PSUM") as psY,
                    tc.tile_pool(name="psT", bufs=1, space="PSUM") as psT,
                    nc.named_scope("ph3_attn"),
                ):
                    for b in range(B):
                        # joint v natural [kv, h, {v(64)|ones(64)}]: the
                        # ones columns make the AV matmul emit softmax
                        # sums on PSUM rows 64..127.
                        vn = vnp.tile([P, 16, 2, P], BF16, tag="vn")
                        if b < 2:  # each of the 2 pool bufs, once
                            nc.vector.tensor_copy(
                                vn[:, :, 0, HD:P], ones_half[:])
                            nc.vector.tensor_copy(
                                vn[:, :, 1, HD:P], ones_half[:])
                        for kq in range(4):  # 4 joint transposes -> 1 evict
                            pt = psT.tile([P, 4, P], BF16, tag="vtp")
                            for k2 in range(4):
                                kb = kq * 4 + k2
                                nc.tensor.transpose(
                                    pt[:, k2, :],
                                    vT[:, b * 4 + kb // 4,
                                       (kb % 4) * P : (kb % 4 + 1) * P],
                                    ident_bf[:],
                                )
                            # pt cols: [h0 dims 64 | h1 dims 64]
                            nc.vector.tensor_copy(
                                vn[:, kq * 4 : (kq + 1) * 4, :, 0:HD],
                                pt[:].rearrange("p t (h c) -> p t h c", h=2),
                            )
                        for tq in range(4):
                            nkv = 4 * (tq + 1)
                            py = psY.tile([P, 2, 512], F32, tag="py")
                            pend = []  # pipelined AV emission
                            for kb in range(nkv):
                                diag = kb >= 4 * tq
                                ps = psQK.tile([P, 2, 512], F32, tag="qk")
                                for h in range(2):
                                    hp = h * HD
                                    nc.tensor.matmul(
                                        ps[:, h, :],
                                        kT[hp : hp + HD, b * 4 + kb // 4,
                                           (kb % 4) * P : (kb % 4 + 1) * P],
                                        qT[hp : hp + HD, b * 4 + tq, :],
                                        start=True, stop=True,
                                    )
                                ex = ph3.tile([P, 2, 512], BF16, tag="ex")
                                nc.scalar.activation(
                                    ex[:], ps[:], Act.Exp, scale=1.0 / np.sqrt(HD)
                                )
                                if diag:  # zero the causal-invalid region
                                    nc.vector.tensor_tensor(
                                        ex[:], ex[:], mask01[:, kb - 4 * tq, :, :],
                                        op=Alu.mult,
                                    )
                                pend.append((kb, ex))
                                if len(pend) > 1:  # one score block ahead
                                    kb0, e0 = pend.pop(0)
                                    for h in range(2):
                                        nc.tensor.matmul(
                                            py[:, h, :],
                                            vn[:, kb0, h, :],
                                            e0[:, h, :],
                                            start=(kb0 == 0), stop=False,
                                        )
                            while pend:
                                kb0, e0 = pend.pop(0)
                                for h in range(2):
                                    nc.tensor.matmul(
                                        py[:, h, :],
                                        vn[:, kb0, h, :],
                                        e0[:, h, :],
                                        start=(kb0 == 0),
                                        stop=(not pend),
                                    )
                            # normalize: rows 64..127 hold the softmax
                            # sums; 1/s = exp(-ln(s)) on ACT (DVE
                            # reciprocal is ~16x slower)
                            for h in range(2):
                                lns = ph3s.tile([HD, 512], F32, tag="lns")
                                nc.scalar.activation(
                                    lns[:], py[HD:P, h, :], Act.Ln)
                                rec = ph3s.tile([HD, 512], F32, tag="rec")
                                nc.scalar.activation(
                                    rec[:], lns[:], Act.Exp, scale=-1.0)
                                yt = ph3s.tile([HD, 512], BF16, tag="yt")
                                nc.vector.tensor_tensor(
                                    yt[:], py[0:HD, h, :], rec[:], op=Alu.mult
                                )
                                # tq covers strips r = 2tq, 2tq+1
                                ysv = y_send[b].rearrange("r h p t -> h p r t")
                                nc.sync.dma_start(
                                    ysv[h, :, 2 * tq : 2 * tq + 2, :],
                                    yt[:].rearrange("p (r t) -> p r t", r=2),
                                )
                        with nc.named_scope(f"cc_a2a_y{b}"):
                            nc.gpsimd.collective_compute(
                                "AllToAll", Alu.bypass, replica_groups=groups,
                                ins=[y_send[b][:]], outs=[y_recv[b][:]],
                            )

            # =========================================================
            # Phase 4: proj + residual + LN2 (own tokens) -> h2^T (SBUF)
            # own t-tile t = batch t//2, strip off (t%2)*128
            # =========================================================
            with tc.tile_pool(name="keep", bufs=1) as keep:
                h2T = keep.tile([P, 8, 8, P], BF16)  # [p, dblk, t, j]
                x1_sb = keep.tile([P, 8, D], F32)    # [p, t, d]
                wfc2_sb = keep.tile([P, DFF // P, D], BF16)
                nc.sync.dma_start(
                    wfc2_sb[:], wfc2.rearrange("(kb p) n -> p kb n", p=P)
                )
                with (
                    tc.tile_pool(name="ph4", bufs=2) as ph4,
                    tc.tile_pool(name="wpp", bufs=1) as wpp,
                    tc.tile_pool(name="psC", bufs=4, space="PSUM") as psC,
                    tc.tile_pool(name="psD", bufs=2, space="PSUM") as psD,
                    nc.named_scope("ph4_proj_ln2"),
                ):
                    wproj_sb = wpp.tile([P, 8, D], BF16)
                    nc.sync.dma_start(
                        wproj_sb[:], wproj.rearrange("(ko p) n -> p ko n", p=P)
                    )
                    # y_recv[b][src, h, p, t]: ydim = 128*src + 64*h + p
                    yrv = [y_recv[b].rearrange("s h p t -> h p s t")
                           for b in range(B)]
                    for t in range(8):
                        yv = yrv[t // 2]
                        off = (t % 2) * P
                        yt_own = ph4.tile([P, 8, P], BF16, tag="ytown")
                        nc.sync.dma_start(
                            yt_own[0:HD, :, :], yv[0][:, :, off : off + P]
                        )
                        nc.sync.dma_start(
                            yt_own[HD:P, :, :], yv[1][:, :, off : off + P]
                        )
                        xt = ph4.tile([P, D], F32, tag="xt4")
                        nc.sync.dma_start(xt[:], x_own[t * P : (t + 1) * P, :])
                        for n in range(2):
                            pp = psC.tile([P, 512], F32, tag="pj")
                            if use_bproj:
                                nc.tensor.matmul(
                                    pp[:], ones_b[:, :P],
                                    bproj_sb[:, n * 512 : (n + 1) * 512],
                                    start=True, stop=False,
                                )
                            for ko in range(8):
                                nc.tensor.matmul(
                                    pp[:], yt_own[:, ko, :],
                                    wproj_sb[:, ko, n * 512 : (n + 1) * 512],
                                    start=(ko == 0 and not use_bproj),
                                    stop=(ko == 7),
                                )
                            nc.vector.tensor_tensor(
                                x1_sb[:, t, n * 512 : (n + 1) * 512], pp[:],
                                xt[:, n * 512 : (n + 1) * 512], op=Alu.add,
                            )
                        layernorm_tile(
                            ph4, x1_sb[:, t, :], ln2w_sb, ln2b_sb, psD, h2T, t
                        )

                # =====================================================
                # Phase 5: MLP (own tokens, 2 groups of 512, bf16)
                # =====================================================
                with (
                    tc.tile_pool(name="mt", bufs=1) as mtp,
                    tc.tile_pool(name="ph5", bufs=3) as ph5,
                    tc.tile_pool(name="psM", bufs=2, space="PSUM") as psM,
                    tc.tile_pool(name="psO", bufs=1, space="PSUM") as psO,
                    nc.named_scope("ph5_mlp"),
                ):
                    wfcv = wfc.rearrange("(ko p) n -> p ko n", p=P)
                    for g in range(2):
                        mT = mtp.tile([P, DFF // P, 512], BF16, tag="mt")
                        for kb in range(DFF // P):
                            wt = ph5.tile([P, 8, P], BF16, tag="wfct")
                            nc.sync.dma_start(
                                wt[:], wfcv[:, :, kb * P : (kb + 1) * P]
                            )
                            pm = psM.tile([P, 512], F32, tag="pm")
                            for ko in range(8):
                                nc.tensor.matmul(
                                    pm[:], wt[:, ko, :],
                                    h2T[:, ko, 4 * g : 4 * g + 4, :],
                                    start=(ko == 0), stop=(ko == 7),
                                )
                            nc.scalar.activation(
                                mT[:, kb, :], pm[:], Act.Gelu,
                                bias=bfc_sb[:, kb : kb + 1],
                            )
                        for n2 in range(2):
                            pos = [
                                psO.tile([P, 512], F32, tag=f"po{t2}",
                                         name=f"po_{g}_{n2}_{t2}")
                                for t2 in range(4)
                            ]
                            if use_bfc2:
                                for t2 in range(4):
                                    nc.tensor.matmul(
                                        pos[t2][:], ones_b[:, :P],
                                        bfc2_sb[:, n2 * 512 : (n2 + 1) * 512],
                                        start=True, stop=False,
                                    )
                            for kb in range(DFF // P):
                                for t2 in range(4):
                                    nc.tensor.matmul(
                                        pos[t2][:], mT[:, kb, t2 * P : (t2 + 1) * P],
                                        wfc2_sb[:, kb, n2 * 512 : (n2 + 1) * 512],
                                        start=(kb == 0 and not use_bfc2),
                                        stop=(kb == DFF // P - 1),
                                    )
                            for t2 in range(4):
                                t = 4 * g + t2
                                ot = ph5.tile([P, 512], F32, tag="ot")
                                nc.vector.tensor_tensor(
                                    ot[:], pos[t2][:],
                                    x1_sb[:, t, n2 * 512 : (n2 + 1) * 512],
                                    op=Alu.add,
                                )
                                nc.sync.dma_start(
                                    out_own[t * P : (t + 1) * P,
                                            n2 * 512 : (n2 + 1) * 512],
                                    ot[:],
                                )

    nc.compile()
    return nc


_NC_CACHE = {}


def kernel(x, ln1_w, ln1_b, ln2_w, ln2_b, w_attn, b_attn, w_proj, b_proj,
           w_fc, b_fc, w_fc2, b_fc2):
    f = np.ascontiguousarray
    x = np.asarray(x, np.float32)
    w_attn = np.asarray(w_attn, np.float32)
    b_attn = np.asarray(b_attn, np.float32)
    b_proj = np.asarray(b_proj, np.float32)
    b_fc2 = np.asarray(b_fc2, np.float32)

    key = (bool(np.any(b_proj)), bool(np.any(b_fc2)))
    if key not in _NC_CACHE:
        _NC_CACHE[key] = build(use_bproj=key[0], use_bfc2=key[1])
    nc = _NC_CACHE[key]
    global _NC_LAST
    _NC_LAST = nc

    # striped ownership: core c owns strip c of every batch
    xs = x.reshape(B, NCORES, STR, D)  # [b, strip, 256, d]

    def col(v, c):  # [128, 1] bias slice
        return f(np.asarray(v, np.float32)[c * P : (c + 1) * P].reshape(P, 1))

    def strip(v):  # [1024] -> [128, 8] with [p, a] = v[a*128 + p]
        return f(np.asarray(v, np.float32).reshape(-1, P).T)

    def bf(v):
        return f(np.asarray(v, np.float32).astype(BF))

    in_maps = []
    for c in range(NCORES):
        in_maps.append({
            "x_own": f(xs[:, c].reshape(TOWN, D)),
            "wq": bf(w_attn[:, P * c : P * (c + 1)]),
            "wk": bf(w_attn[:, D + P * c : D + P * (c + 1)]),
            "wv": bf(w_attn[:, 2 * D + P * c : 2 * D + P * (c + 1)]),
            "bq": col(b_attn, c),
            "bk": col(b_attn[D:], c),
            "bv": col(b_attn[2 * D:], c),
            "ln1w": strip(ln1_w), "ln1b": strip(ln1_b),
            "ln2w": strip(ln2_w), "ln2b": strip(ln2_b),
            "wproj": bf(w_proj),
            "bproj": bf(b_proj.reshape(1, D)),
            "wfc": bf(w_fc),
            "bfc": strip(b_fc),
            "wfc2": bf(w_fc2),
            "bfc2": bf(b_fc2.reshape(1, D)),
        })

    global _last_in_maps
    _last_in_maps = in_maps
    res = run_bass_kernel_spmd(nc, in_maps, core_ids=list(range(NCORES)))
    # reassemble: core c's rows are [b, strip c] pieces
    out = np.empty((B, NCORES, STR, D), np.float32)
    for c in range(NCORES):
        out[:, c] = res.results[c]["out"].reshape(B, STR, D)
    return out.reshape(B, T, D)


_NC_LAST = None
_last_in_maps = None


# revision 14
# speedup vs baseline: 1.1432x; 1.1432x over previous
"""Trainium2 Bass kernel for a GPT-style transformer block.

Reference computation (B=4, T=2048, d=1024, 16 heads, dff=4096, fp32):
    h  = LN1(x);  qkv = h @ w_attn + b_attn
    y  = causal_attention(q, k, v);  x1 = x + y @ w_proj + b_proj
    h2 = LN2(x1); out = x1 + gelu(h2 @ w_fc + b_fc) @ w_fc2 + b_fc2

Sharding over 8 NeuronCores (one trn2 chip), STRIPED token ownership:
  core c owns token strip [2048*b + 256*c, +256) of every batch b (1024
  tokens total).  This alignment makes each LN1 AllGather chunk ci
  deliver exactly batch ci's tokens, and lets the per-head attention
  output redistribute with one small AllToAll per batch, pipelined
  behind the next batch's attention.

  - ph1: LN1 over own tokens in 4 chunks of 256; each chunk's h^T shard
    (bf16) AllGathers as soon as it is ready.  A 1-byte dummy AllGather
    at kernel start absorbs the ~50us collective-init barrier.
  - ph2: per gathered chunk, q^T/k^T/v^T (bf16, both heads stacked
    on 128 partitions) for this core's 2 heads over that batch.
  - ph3: causal attention, batch-major.  Scores for the two heads run
    CONCURRENTLY as row-tiled K=64 matmuls (rows 0-63 / 64-127 of the
    PE array) into separate PSUM banks.  exp on ACT; causal masking by
    a 0/1 bf16 multiply on DVE (diagonal blocks only); softmax
    denominator comes from 64 ones-columns appended to v, and the
    normalization reciprocal is computed as exp(-ln(s)) on ACT (DVE
    reciprocal is ~16x slower).  After each batch, one AllToAll
    redistributes y to token owners, overlapped with the next batch.
  - ph4/ph5: token-parallel proj+residual+LN2 and MLP with full-width
    weights, as in the reference.

Matmul-shape notes: matmul time = moving-free-dim cycles (independent
of K and M), so the 64-ones columns and K=64 padding are free; what
matters is slot count, which row-tiling halves for scores.  Bias
matmuls (ones-row trick) are skipped at build time when the bias
vectors are all zero (they are, for this problem's inputs).
"""

import sys

import numpy as np
import ml_dtypes

sys.path.insert(0, "/opt/trn_rl_repo")

import concourse.bass as bass  # noqa: E402
import concourse.mybir as mybir  # noqa: E402
import concourse.tile as tile  # noqa: E402
from concourse import bacc  # noqa: E402
from concourse.bass_utils import run_bass_kernel_spmd  # noqa: E402
from concourse.masks import make_identity  # noqa: E402

B, T, D, H, HD, DFF = 4, 2048, 1024, 16, 64, 4096
EPS = 1e-5
NCORES = 8
TOK = B * T            # 8192 flattened tokens
TOWN = TOK // NCORES   # 1024 tokens owned per core
STR = 256              # per-batch strip owned per core
P = 128
F32 = mybir.dt.float32
BF16 = mybir.dt.bfloat16
Act = mybir.ActivationFunctionType
Alu = mybir.AluOpType
AX = mybir.AxisListType
BF = ml_dtypes.bfloat16


def build(use_bproj=True, use_bfc2=True):
    nc = bacc.Bacc("TRN2", target_bir_lowering=False, debug=False, num_devices=NCORES)

    def inp(name, shape, dt=F32):
        return nc.dram_tensor(name, shape, dt, kind="ExternalInput").ap()

    x_own = inp("x_own", [TOWN, D])
    wq = inp("wq", [D, P], BF16)
    wk = inp("wk", [D, P], BF16)
    wv = inp("wv", [D, P], BF16)
    bq = inp("bq", [P, 1])
    bk = inp("bk", [P, 1])
    bv = inp("bv", [P, 1])
    ln1w = inp("ln1w", [P, 8])
    ln1b = inp("ln1b", [P, 8])
    ln2w = inp("ln2w", [P, 8])
    ln2b = inp("ln2b", [P, 8])
    wproj = inp("wproj", [D, D], BF16)
    bproj = inp("bproj", [1, D], BF16)
    wfc = inp("wfc", [D, DFF], BF16)
    bfc = inp("bfc", [P, DFF // P])
    wfc2 = inp("wfc2", [DFF, D], BF16)
    bfc2 = inp("bfc2", [1, D], BF16)
    out_own = nc.dram_tensor("out", [TOWN, D], F32, kind="ExternalOutput").ap()

    groups = [list(range(NCORES))]

    with tile.TileContext(nc) as tc:
        with (
            tc.tile_pool(name="const", bufs=1) as cst,
            tc.tile_pool(name="dram", bufs=1, space="DRAM") as dram,
        ):
            # ---------------- constants ----------------
            ident = cst.tile([P, P], F32)
            make_identity(nc, ident)
            ident_bf = cst.tile([P, P], BF16)
            make_identity(nc, ident_bf)
            ones_b = None
            if use_bproj or use_bfc2:
                ones_f = cst.tile([1, P], F32)
                nc.vector.memset(ones_f[:], 1.0)
                ones_b = cst.tile([1, P], BF16)
                nc.scalar.copy(ones_b[:], ones_f[:])
            ln1w_sb = cst.tile([P, 8], F32)
            nc.sync.dma_start(ln1w_sb[:], ln1w)
            ln1b_sb = cst.tile([P, 8], F32)
            nc.sync.dma_start(ln1b_sb[:], ln1b)
            ln2w_sb = cst.tile([P, 8], F32)
            nc.sync.dma_start(ln2w_sb[:], ln2w)
            ln2b_sb = cst.tile([P, 8], F32)
            nc.sync.dma_start(ln2b_sb[:], ln2b)
            bq_sb = cst.tile([P, 1], F32)
            nc.sync.dma_start(bq_sb[:], bq)
            bk_sb = cst.tile([P, 1], F32)
            nc.sync.dma_start(bk_sb[:], bk)
            bv_sb = cst.tile([P, 1], F32)
            nc.sync.dma_start(bv_sb[:], bv)
            bproj_sb = None
            if use_bproj:
                bproj_sb = cst.tile([1, D], BF16)
                nc.sync.dma_start(bproj_sb[:], bproj)
            bfc_sb = cst.tile([P, DFF // P], F32)
            nc.sync.dma_start(bfc_sb[:], bfc)
            bfc2_sb = None
            if use_bfc2:
                bfc2_sb = cst.tile([1, D], BF16)
                nc.sync.dma_start(bfc2_sb[:], bfc2)
            # 0/1 causal masks for the 4 diagonal offsets, replicated for
            # both heads: mask01[s][i, h, j] = 1 if i <= j - 128*s else 0
            mask01 = cst.tile([P, 4, 2, 512], BF16)
            nc.vector.memset(mask01[:], 1.0)
            for s in range(4):
                for h in range(2):
                    nc.gpsimd.affine_select(
                        out=mask01[:, s, h, :],
                        in_=mask01[:, s, h, :],
                        pattern=[[1, 512]],
                        channel_multiplier=-1,
                        base=-128 * s,
                        compare_op=Alu.is_ge,
                        fill=0.0,
                    )

            # DRAM intermediates.
            # hT chunk ci: own tokens [512ci, 512ci+512) -> gathered chunk
            # ci holds batches {2ci, 2ci+1} as [8 strips, ...].
            NCH = 2
            CHT = TOWN // NCH  # own tokens per chunk
            hT_dram = [dram.tile([D, CHT], BF16, name=f"hq{i}") for i in range(NCH)]
            hT_full = [dram.tile([NCORES * D, CHT], BF16, addr_space="Shared",
                                 name=f"hfq{i}") for i in range(NCH)]
            # per-batch y AllToAll: slice r = my 2 heads' y for core r's
            # strip of this batch.
            y_send = [dram.tile([NCORES, 2, HD, STR], BF16, name=f"ys{b}")
                      for b in range(B)]
            y_recv = [dram.tile([NCORES, 2, HD, STR], BF16, name=f"yr{b}")
                      for b in range(B)]

            # =========================================================
            # Phase 1: LN1 over own tokens, 4 chunks -> AllGather each
            # =========================================================
            def layernorm_tile(pool, xt, w_sb, b_sb, ps_pool, dstT, t):
                """LN a [128, D] token tile and write transposed blocks
                (with gamma/beta applied) into dstT[:, dblk, t, :] (bf16)."""
                ssum = pool.tile([P, 1], F32, tag="ssum")
                nc.vector.reduce_sum(ssum[:], xt[:], axis=AX.X)
                mean = pool.tile([P, 1], F32, tag="mean")
                nc.scalar.mul(mean[:], ssum[:], 1.0 / D)
                sq = pool.tile([P, D], F32, tag="sq")
                sumsq = pool.tile([P, 1], F32, tag="sumsq")
                nc.scalar.activation(sq[:], xt[:], Act.Square, accum_out=sumsq[:])
                msq = pool.tile([P, 1], F32, tag="msq")
                nc.vector.tensor_tensor(msq[:], mean[:], mean[:], op=Alu.mult)
                var = pool.tile([P, 1], F32, tag="var")
                nc.vector.tensor_scalar(var[:], sumsq[:], 1.0 / D, EPS, Alu.mult, Alu.add)
                nc.vector.tensor_tensor(var[:], var[:], msq[:], op=Alu.subtract)
                rinv = pool.tile([P, 1], F32, tag="rinv")
                nc.vector.reciprocal(rinv[:], var[:])
                rstd = pool.tile([P, 1], F32, tag="rstd")
                nc.scalar.sqrt(rstd[:], rinv[:])
                hh = pool.tile([P, D], F32, tag="hh")
                nc.vector.tensor_scalar(
                    hh[:], xt[:], mean[:], rstd[:], Alu.subtract, Alu.mult
                )
                for dblk in range(8):
                    pt = ps_pool.tile([P, P], F32, tag="lnt")
                    nc.tensor.transpose(pt[:], hh[:, dblk * P : (dblk + 1) * P], ident[:])
                    nc.scalar.activation(
                        dstT[:, dblk, t, :],
                        pt[:],
                        Act.Identity,
                        bias=b_sb[:, dblk : dblk + 1],
                        scale=w_sb[:, dblk : dblk + 1],
                    )

            with (
                tc.tile_pool(name="ph1", bufs=2) as ph1,
                tc.tile_pool(name="ph1T", bufs=1) as ph1T,
                tc.tile_pool(name="psA", bufs=2, space="PSUM") as psA,
                nc.named_scope("ph1_ln1"),
            ):
                hT_asm = ph1T.tile([P, 8, 8, P], BF16)  # [p, dblk, t, j]
                TPC = 8 // NCH  # token tiles per chunk
                for ci in range(NCH):
                    for t in range(TPC * ci, TPC * ci + TPC):
                        xt = ph1.tile([P, D], F32, tag="xt")
                        nc.sync.dma_start(xt[:], x_own[t * P : (t + 1) * P, :])
                        layernorm_tile(ph1, xt, ln1w_sb, ln1b_sb, psA, hT_asm, t)
                    hTv = hT_dram[ci].rearrange("(dblk p) t -> p dblk t", p=P)
                    for dblk in range(8):
                        nc.sync.dma_start(
                            hTv[:, dblk, :],
                            hT_asm[:, dblk, TPC * ci : TPC * ci + TPC, :],
                        )
                    nc.gpsimd.collective_compute(
                        "AllGather", Alu.bypass, replica_groups=groups,
                        ins=[hT_dram[ci][:]], outs=[hT_full[ci][:]],
                    )

            # weights for ph4/ph5, prefetched during the AllGather wait
            # window (DMA engines are idle there)
            wpre_cm = tc.tile_pool(name="wpre", bufs=1)
            wpre = wpre_cm.__enter__()
            wproj_sb = wpre.tile([P, 8, D], BF16)
            nc.sync.dma_start(
                wproj_sb[:], wproj.rearrange("(ko p) n -> p ko n", p=P)
            )
            wfc2_sb = wpre.tile([P, DFF // P, D], BF16)
            nc.sync.dma_start(
                wfc2_sb[:], wfc2.rearrange("(kb p) n -> p kb n", p=P)
            )

            # =========================================================
            # Phase 2: q^T, k^T (bf16, both heads stacked on 128
            # partitions) and v-natural (vn, with 64 ones-columns per
            # head) per gathered chunk.  Gathered chunk ci strip rr
            # holds batches {2ci, 2ci+1}: local t in [0,512) -> batch
            # 2ci + t//256, global tok 2048*(2ci+t//256) + 256*rr + t%256
            # =========================================================
            with tc.tile_pool(name="qkv", bufs=1) as qkvp:
                qT = qkvp.tile([P, 16, 512], BF16)
                kT = qkvp.tile([P, 16, 512], BF16)
                # vn[kv_p, g, h, 0:64] = v dims of head h for kv block g;
                # cols 64:128 = 1.0 (softmax-sum columns for the AV mm)
                vn = qkvp.tile([P, 64, 2, P], BF16)
                nc.vector.memset(vn[:, :, :, HD:P], 1.0)
                with (
                    tc.tile_pool(name="wqkv", bufs=1) as wp,
                    tc.tile_pool(name="ph2", bufs=3) as ph2,
                    tc.tile_pool(name="psB", bufs=3, space="PSUM") as psB,
                    tc.tile_pool(name="psBT", bufs=2, space="PSUM") as psBT,
                    nc.named_scope("ph2_qkv"),
                ):
                    wq_sb = wp.tile([P, 8, P], BF16)
                    nc.sync.dma_start(wq_sb[:], wq.rearrange("(ko p) m -> p ko m", p=P))
                    wk_sb = wp.tile([P, 8, P], BF16)
                    nc.sync.dma_start(wk_sb[:], wk.rearrange("(ko p) m -> p ko m", p=P))
                    wv_sb = wp.tile([P, 8, P], BF16)
                    nc.sync.dma_start(wv_sb[:], wv.rearrange("(ko p) m -> p ko m", p=P))
                    hfvs = [hq.rearrange("(r ko p) t -> r p ko t", p=P, ko=8)
                            for hq in hT_full]
                    for ci in range(NCH):
                        hfv = hfvs[ci]
                        for rr in range(8):
                            ht = ph2.tile([P, 8, CHT], BF16, tag="ht")
                            nc.sync.dma_start(ht[:], hfv[rr])
                            # two 256-token halves -> two tile16 slots
                            t16a = 8 * ci + rr // 2
                            co = (rr % 2) * STR
                            sls = [
                                (slice(None), t16a, slice(co, co + STR)),
                                (slice(None), t16a + 4, slice(co, co + STR)),
                            ]
                            for wi, (w_sb, b_sb, dstT) in enumerate(
                                ((wq_sb, bq_sb, qT), (wk_sb, bk_sb, kT),
                                 (wv_sb, bv_sb, None))
                            ):
                                ps = psB.tile([P, CHT], F32, tag="qkvps")
                                for ko in range(8):
                                    nc.tensor.matmul(
                                        ps[:], w_sb[:, ko, :], ht[:, ko, :],
                                        start=(ko == 0), stop=(ko == 7),
                                    )
                                if dstT is not None:
                                    for half in range(2):
                                        nc.scalar.activation(
                                            dstT[sls[half]],
                                            ps[:, half * STR : half * STR + STR],
                                            Act.Identity, bias=b_sb[:],
                                        )
                                else:
                                    # v: bias, transpose to natural, pack
                                    # into vn (dims on cols, split heads)
                                    vtmp = ph2.tile([P, CHT], BF16, tag="vtmp")
                                    nc.scalar.activation(
                                        vtmp[:], ps[:], Act.Identity,
                                        bias=b_sb[:],
                                    )
                                    for jp in range(2):  # pairs of kv blocks
                                        pt = psBT.tile([P, 2, P], BF16, tag="vt")
                                        for j2 in range(2):
                                            j = jp * 2 + j2
                                            nc.tensor.transpose(
                                                pt[:, j2, :],
                                                vtmp[:, j * P : (j + 1) * P],
                                                ident_bf[:],
                                            )
                                        g0 = 32 * ci + 16 * jp + 2 * rr
                                        nc.vector.tensor_copy(
                                            vn[:, g0 : g0 + 2, :, 0:HD],
                                            pt[:].rearrange(
                                                "p j (h c) -> p j h c", h=2),
                                        )

                # =====================================================
                # Phase 3: causal attention, batch-major; both heads
                # concurrently via row-tiled K=64 score matmuls.
                # =====================================================
                with (
                    tc.tile_pool(name="ph3", bufs=4) as ph3,
                    tc.tile_pool(name="ph3s", bufs=3) as ph3s,
                    tc.tile_pool(name="psQK", bufs=2, space="PSUM") as psQK,
                    tc.tile_pool(name="psY", bufs=2, space="PSUM") as psY,
                    nc.named_scope("ph3_attn"),
                ):
                    for b in range(B):
                        for tq in range(4):
                            nkv = 4 * (tq + 1)
                            py = psY.tile([P, 2, 512], F32, tag="py")
                            pend = []  # pipelined AV emission
                            for kb in range(nkv):
                                diag = kb >= 4 * tq
                                ps = psQK.tile([P, 2, 512], F32, tag="qk")
                                for h in range(2):
                                    hp = h * HD
                                    nc.tensor.matmul(
                                        ps[:, h, :],
                                        kT[hp : hp + HD, b * 4 + kb // 4,
                                           (kb % 4) * P : (kb % 4 + 1) * P],
                                        qT[hp : hp + HD, b * 4 + tq, :],
                                        start=True, stop=True,
                                    )
                                ex = ph3.tile([P, 2, 512], BF16, tag="ex")
                                nc.scalar.activation(
                                    ex[:], ps[:], Act.Exp, scale=1.0 / np.sqrt(HD)
                                )
                                if diag:  # zero the causal-invalid region
                                    nc.vector.tensor_tensor(
                                        ex[:], ex[:], mask01[:, kb - 4 * tq, :, :],
                                        op=Alu.mult,
                                    )
                                pend.append((kb, ex))
                                if len(pend) > 1:  # one score block ahead
                                    kb0, e0 = pend.pop(0)
                                    for h in range(2):
                                        nc.tensor.matmul(
                                            py[:, h, :],
                                            vn[:, 16 * b + kb0, h, :],
                                            e0[:, h, :],
                                            start=(kb0 == 0), stop=False,
                                        )
                            while pend:
                                kb0, e0 = pend.pop(0)
                                for h in range(2):
                                    nc.tensor.matmul(
                                        py[:, h, :],
                                        vn[:, 16 * b + kb0, h, :],
                                        e0[:, h, :],
                                        start=(kb0 == 0),
                                        stop=(not pend),
                                    )
                            # normalize: rows 64..127 hold the softmax
                            # sums (replicated by the ones columns)
                            for h in range(2):
                                rec = ph3s.tile([HD, 512], F32, tag="rec")
                                nc.vector.reciprocal(rec[:], py[HD:P, h, :])
                                yt = ph3s.tile([HD, 512], BF16, tag="yt")
                                nc.vector.tensor_tensor(
                                    yt[:], py[0:HD, h, :], rec[:], op=Alu.mult
                                )
                                # tq covers strips r = 2tq, 2tq+1
                                ysv = y_send[b].rearrange("r h p t -> h p r t")
                                nc.sync.dma_start(
                                    ysv[h, :, 2 * tq : 2 * tq + 2, :],
                                    yt[:].rearrange("p (r t) -> p r t", r=2),
                                )
                        with nc.named_scope(f"cc_a2a_y{b}"):
                            nc.gpsimd.collective_compute(
                                "AllToAll", Alu.bypass, replica_groups=groups,
                                ins=[y_send[b][:]], outs=[y_recv[b][:]],
                            )

            # =========================================================
            # Phase 4: proj + residual + LN2 (own tokens) -> h2^T (SBUF)
            # own t-tile t = batch t//2, strip off (t%2)*128
            # =========================================================
            with tc.tile_pool(name="keep", bufs=1) as keep:
                h2T = keep.tile([P, 8, 8, P], BF16)  # [p, dblk, t, j]
                x1_sb = keep.tile([P, 8, D], F32)    # [p, t, d]
                with (
                    tc.tile_pool(name="ph4", bufs=2) as ph4,
                    tc.tile_pool(name="psC", bufs=4, space="PSUM") as psC,
                    tc.tile_pool(name="psD", bufs=2, space="PSUM") as psD,
                    nc.named_scope("ph4_proj_ln2"),
                ):
                    # y_recv[b][src, h, p, t]: ydim = 128*src + 64*h + p
                    yrv = [y_recv[b].rearrange("s h p t -> h p s t")
                           for b in range(B)]
                    for t in range(8):
                        yv = yrv[t // 2]
                        off = (t % 2) * P
                        yt_own = ph4.tile([P, 8, P], BF16, tag="ytown")
                        nc.sync.dma_start(
                            yt_own[0:HD, :, :], yv[0][:, :, off : off + P]
                        )
                        nc.sync.dma_start(
                            yt_own[HD:P, :, :], yv[1][:, :, off : off + P]
                        )
                        xt = ph4.tile([P, D], F32, tag="xt4")
                        nc.sync.dma_start(xt[:], x_own[t * P : (t + 1) * P, :])
                        for n in range(2):
                            pp = psC.tile([P, 512], F32, tag="pj")
                            if use_bproj:
                                nc.tensor.matmul(
                                    pp[:], ones_b[:, :P],
                                    bproj_sb[:, n * 512 : (n + 1) * 512],
                                    start=True, stop=False,
                                )
                            for ko in range(8):
                                nc.tensor.matmul(
                                    pp[:], yt_own[:, ko, :],
                                    wproj_sb[:, ko, n * 512 : (n + 1) * 512],
                                    start=(ko == 0 and not use_bproj),
                                    stop=(ko == 7),
                                )
                            nc.vector.tensor_tensor(
                                x1_sb[:, t, n * 512 : (n + 1) * 512], pp[:],
                                xt[:, n * 512 : (n + 1) * 512], op=Alu.add,
                            )
                        layernorm_tile(
                            ph4, x1_sb[:, t, :], ln2w_sb, ln2b_sb, psD, h2T, t
                        )

                # =====================================================
                # Phase 5: MLP (own tokens, 2 groups of 512, bf16)
                # =====================================================
                with (
                    tc.tile_pool(name="mt", bufs=1) as mtp,
                    tc.tile_pool(name="ph5", bufs=3) as ph5,
                    tc.tile_pool(name="psM", bufs=2, space="PSUM") as psM,
                    tc.tile_pool(name="psO", bufs=1, space="PSUM") as psO,
                    nc.named_scope("ph5_mlp"),
                ):
                    wfcv = wfc.rearrange("(ko p) n -> p ko n", p=P)
                    for g in range(2):
                        mT = mtp.tile([P, DFF // P, 512], BF16, tag="mt")
                        for kb in range(DFF // P):
                            wt = ph5.tile([P, 8, P], BF16, tag="wfct")
                            nc.sync.dma_start(
                                wt[:], wfcv[:, :, kb * P : (kb + 1) * P]
                            )
                            pm = psM.tile([P, 512], F32, tag="pm")
                            for ko in range(8):
                                nc.tensor.matmul(
                                    pm[:], wt[:, ko, :],
                                    h2T[:, ko, 4 * g : 4 * g + 4, :],
                                    start=(ko == 0), stop=(ko == 7),
                                )
                            nc.scalar.activation(
                                mT[:, kb, :], pm[:], Act.Gelu,
                                bias=bfc_sb[:, kb : kb + 1],
                            )
                        for n2 in range(2):
                            pos = [
                                psO.tile([P, 512], F32, tag=f"po{t2}",
                                         name=f"po_{g}_{n2}_{t2}")
                                for t2 in range(4)
                            ]
                            if use_bfc2:
                                for t2 in range(4):
                                    nc.tensor.matmul(
                                        pos[t2][:], ones_b[:, :P],
                                        bfc2_sb[:, n2 * 512 : (n2 + 1) * 512],
                                        start=True, stop=False,
                                    )
                            for kb in range(DFF // P):
                                for t2 in range(4):
                                    nc.tensor.matmul(
                                        pos[t2][:], mT[:, kb, t2 * P : (t2 + 1) * P],
                                        wfc2_sb[:, kb, n2 * 512 : (n2 + 1) * 512],
                                        start=(kb == 0 and not use_bfc2),
                                        stop=(kb == DFF // P - 1),
                                    )
                            for t2 in range(4):
                                t = 4 * g + t2
                                ot = ph5.tile([P, 512], F32, tag="ot")
                                nc.vector.tensor_tensor(
                                    ot[:], pos[t2][:],
                                    x1_sb[:, t, n2 * 512 : (n2 + 1) * 512],
                                    op=Alu.add,
                                )
                                nc.sync.dma_start(
                                    out_own[t * P : (t + 1) * P,
                                            n2 * 512 : (n2 + 1) * 512],
                                    ot[:],
                                )
            wpre_cm.__exit__(None, None, None)

    nc.compile()
    return nc


_NC_CACHE = {}


def kernel(x, ln1_w, ln1_b, ln2_w, ln2_b, w_attn, b_attn, w_proj, b_proj,
           w_fc, b_fc, w_fc2, b_fc2):
    f = np.ascontiguousarray
    x = np.asarray(x, np.float32)
    w_attn = np.asarray(w_attn, np.float32)
    b_attn = np.asarray(b_attn, np.float32)
    b_proj = np.asarray(b_proj, np.float32)
    b_fc2 = np.asarray(b_fc2, np.float32)

    key = (bool(np.any(b_proj)), bool(np.any(b_fc2)))
    if key not in _NC_CACHE:
        _NC_CACHE[key] = build(use_bproj=key[0], use_bfc2=key[1])
    nc = _NC_CACHE[key]
    global _NC_LAST
    _NC_LAST = nc

    # striped ownership: core c owns strip c of every batch
    xs = x.reshape(B, NCORES, STR, D)  # [b, strip, 256, d]

    def col(v, c):  # [128, 1] bias slice
        return f(np.asarray(v, np.float32)[c * P : (c + 1) * P].reshape(P, 1))

    def strip(v):  # [1024] -> [128, 8] with [p, a] = v[a*128 + p]
        return f(np.asarray(v, np.float32).reshape(-1, P).T)

    def bf(v):
        return f(np.asarray(v, np.float32).astype(BF))

    in_maps = []
    for c in range(NCORES):
        in_maps.append({
            "x_own": f(xs[:, c].reshape(TOWN, D)),
            "wq": bf(w_attn[:, P * c : P * (c + 1)]),
            "wk": bf(w_attn[:, D + P * c : D + P * (c + 1)]),
            "wv": bf(w_attn[:, 2 * D + P * c : 2 * D + P * (c + 1)]),
            "bq": col(b_attn, c),
            "bk": col(b_attn[D:], c),
            "bv": col(b_attn[2 * D:], c),
            "ln1w": strip(ln1_w), "ln1b": strip(ln1_b),
            "ln2w": strip(ln2_w), "ln2b": strip(ln2_b),
            "wproj": bf(w_proj),
            "bproj": bf(b_proj.reshape(1, D)),
            "wfc": bf(w_fc),
            "bfc": strip(b_fc),
            "wfc2": bf(w_fc2),
            "bfc2": bf(b_fc2.reshape(1, D)),
        })

    global _last_in_maps
    _last_in_maps = in_maps
    res = run_bass_kernel_spmd(nc, in_maps, core_ids=list(range(NCORES)))
    # reassemble: core c's rows are [b, strip c] pieces
    out = np.empty((B, NCORES, STR, D), np.float32)
    for c in range(NCORES):
        out[:, c] = res.results[c]["out"].reshape(B, STR, D)
    return out.reshape(B, T, D)


_NC_LAST = None
_last_in_maps = None


# revision 17
# speedup vs baseline: 1.1767x; 1.0293x over previous
"""Trainium2 Bass kernel for a GPT-style transformer block.

Reference computation (B=4, T=2048, d=1024, 16 heads, dff=4096, fp32):
    h  = LN1(x);  qkv = h @ w_attn + b_attn
    y  = causal_attention(q, k, v);  x1 = x + y @ w_proj + b_proj
    h2 = LN2(x1); out = x1 + gelu(h2 @ w_fc + b_fc) @ w_fc2 + b_fc2

Sharding over 8 NeuronCores (one trn2 chip), STRIPED token ownership:
  core c owns token strip [2048*b + 256*c, +256) of every batch b (1024
  tokens total).  This alignment makes each LN1 AllGather chunk ci
  deliver exactly batch ci's tokens, and lets the per-head attention
  output redistribute with one small AllToAll per batch, pipelined
  behind the next batch's attention.

  - ph1: LN1 over own tokens in 4 chunks of 256; each chunk's h^T shard
    (bf16) AllGathers as soon as it is ready.  A 1-byte dummy AllGather
    at kernel start absorbs the ~50us collective-init barrier.
  - ph2: per gathered chunk, q^T/k^T/v^T (bf16, both heads stacked
    on 128 partitions) for this core's 2 heads over that batch.
  - ph3: causal attention, batch-major.  Scores for the two heads run
    CONCURRENTLY as row-tiled K=64 matmuls (rows 0-63 / 64-127 of the
    PE array) into separate PSUM banks.  exp on ACT; causal masking by
    a 0/1 bf16 multiply on DVE (diagonal blocks only); softmax
    denominator comes from 64 ones-columns appended to v, and the
    normalization reciprocal is computed as exp(-ln(s)) on ACT (DVE
    reciprocal is ~16x slower).  After each batch, one AllToAll
    redistributes y to token owners, overlapped with the next batch.
  - ph4/ph5: token-parallel proj+residual+LN2 and MLP with full-width
    weights, as in the reference.

Matmul-shape notes: matmul time = moving-free-dim cycles (independent
of K and M), so the 64-ones columns and K=64 padding are free; what
matters is slot count, which row-tiling halves for scores.  Bias
matmuls (ones-row trick) are skipped at build time when the bias
vectors are all zero (they are, for this problem's inputs).
"""

import sys

import numpy as np
import ml_dtypes

sys.path.insert(0, "/opt/trn_rl_repo")

import concourse.bass as bass  # noqa: E402
import concourse.mybir as mybir  # noqa: E402
import concourse.tile as tile  # noqa: E402
from concourse import bacc  # noqa: E402
from concourse.bass_utils import run_bass_kernel_spmd  # noqa: E402
from concourse.masks import make_identity  # noqa: E402

B, T, D, H, HD, DFF = 4, 2048, 1024, 16, 64, 4096
EPS = 1e-5
NCORES = 8
TOK = B * T            # 8192 flattened tokens
TOWN = TOK // NCORES   # 1024 tokens owned per core
STR = 256              # per-batch strip owned per core
P = 128
F32 = mybir.dt.float32
BF16 = mybir.dt.bfloat16
FP8 = mybir.dt.float8e4
Act = mybir.ActivationFunctionType
Alu = mybir.AluOpType
AX = mybir.AxisListType
BF = ml_dtypes.bfloat16

H_FP8 = True        # gather h in fp8e4m3 (halves the AllGather bytes)
USE_DIVIDE = False  # DVE tensor_tensor divide fails neuronxcc codegen
HDT = FP8 if H_FP8 else BF16


def build(use_bproj=True, use_bfc2=True):
    nc = bacc.Bacc("TRN2", target_bir_lowering=False, debug=False, num_devices=NCORES)

    def inp(name, shape, dt=F32):
        return nc.dram_tensor(name, shape, dt, kind="ExternalInput").ap()

    x_own = inp("x_own", [TOWN, D])
    wq = inp("wq", [D, P], BF16)
    wk = inp("wk", [D, P], BF16)
    wv = inp("wv", [D, P], BF16)
    bq = inp("bq", [P, 1])
    bk = inp("bk", [P, 1])
    bv = inp("bv", [P, 1])
    ln1w = inp("ln1w", [P, 8])
    ln1b = inp("ln1b", [P, 8])
    ln2w = inp("ln2w", [P, 8])
    ln2b = inp("ln2b", [P, 8])
    wproj = inp("wproj", [D, D], BF16)
    bproj = inp("bproj", [1, D], BF16)
    wfc = inp("wfc", [D, DFF], BF16)
    bfc = inp("bfc", [P, DFF // P])
    wfc2 = inp("wfc2", [DFF, D], BF16)
    bfc2 = inp("bfc2", [1, D], BF16)
    out_own = nc.dram_tensor("out", [TOWN, D], F32, kind="ExternalOutput").ap()

    groups = [list(range(NCORES))]

    with tile.TileContext(nc) as tc:
        with (
            tc.tile_pool(name="const", bufs=1) as cst,
            tc.tile_pool(name="dram", bufs=1, space="DRAM") as dram,
        ):
            # ---------------- constants ----------------
            ident = cst.tile([P, P], F32)
            make_identity(nc, ident)
            ident_bf = cst.tile([P, P], BF16)
            make_identity(nc, ident_bf)
            ones_b = None
            if use_bproj or use_bfc2:
                ones_f = cst.tile([1, P], F32)
                nc.vector.memset(ones_f[:], 1.0)
                ones_b = cst.tile([1, P], BF16)
                nc.scalar.copy(ones_b[:], ones_f[:])
            ln1w_sb = cst.tile([P, 8], F32)
            nc.sync.dma_start(ln1w_sb[:], ln1w)
            ln1b_sb = cst.tile([P, 8], F32)
            nc.sync.dma_start(ln1b_sb[:], ln1b)
            ln2w_sb = cst.tile([P, 8], F32)
            nc.sync.dma_start(ln2w_sb[:], ln2w)
            ln2b_sb = cst.tile([P, 8], F32)
            nc.sync.dma_start(ln2b_sb[:], ln2b)
            bq_sb = cst.tile([P, 1], F32)
            nc.sync.dma_start(bq_sb[:], bq)
            bk_sb = cst.tile([P, 1], F32)
            nc.sync.dma_start(bk_sb[:], bk)
            bv_sb = cst.tile([P, 1], F32)
            nc.sync.dma_start(bv_sb[:], bv)
            bproj_sb = None
            if use_bproj:
                bproj_sb = cst.tile([1, D], BF16)
                nc.sync.dma_start(bproj_sb[:], bproj)
            bfc_sb = cst.tile([P, DFF // P], F32)
            nc.sync.dma_start(bfc_sb[:], bfc)
            bfc2_sb = None
            if use_bfc2:
                bfc2_sb = cst.tile([1, D], BF16)
                nc.sync.dma_start(bfc2_sb[:], bfc2)
            # 0/1 causal masks for the 4 diagonal offsets, replicated for
            # both heads: mask01[s][i, h, j] = 1 if i <= j - 128*s else 0
            mask01 = cst.tile([P, 4, 2, 512], BF16)
            nc.vector.memset(mask01[:], 1.0)
            for s in range(4):
                for h in range(2):
                    nc.gpsimd.affine_select(
                        out=mask01[:, s, h, :],
                        in_=mask01[:, s, h, :],
                        pattern=[[1, 512]],
                        channel_multiplier=-1,
                        base=-128 * s,
                        compare_op=Alu.is_ge,
                        fill=0.0,
                    )

            # DRAM intermediates.
            # hT chunk ci: own tokens [512ci, 512ci+512) -> gathered chunk
            # ci holds batches {2ci, 2ci+1} as [8 strips, ...].
            NCH = 2
            CHT = TOWN // NCH  # own tokens per chunk
            hT_dram = [dram.tile([D, CHT], HDT, name=f"hq{i}") for i in range(NCH)]
            hT_full = [dram.tile([NCORES * D, CHT], HDT, addr_space="Shared",
                                 name=f"hfq{i}") for i in range(NCH)]
            # per-batch y AllToAll: slice r = my 2 heads' y for core r's
            # strip of this batch.
            y_send = [dram.tile([NCORES, 2, HD, STR], BF16, name=f"ys{b}")
                      for b in range(B)]
            y_recv = [dram.tile([NCORES, 2, HD, STR], BF16, name=f"yr{b}")
                      for b in range(B)]

            # =========================================================
            # Phase 1: LN1 over own tokens, 4 chunks -> AllGather each
            # =========================================================
            def layernorm_tile(pool, xt, w_sb, b_sb, ps_pool, dstT, t):
                """LN a [128, D] token tile and write transposed blocks
                (with gamma/beta applied) into dstT[:, dblk, t, :] (bf16)."""
                ssum = pool.tile([P, 1], F32, tag="ssum")
                nc.vector.reduce_sum(ssum[:], xt[:], axis=AX.X)
                mean = pool.tile([P, 1], F32, tag="mean")
                nc.scalar.mul(mean[:], ssum[:], 1.0 / D)
                sq = pool.tile([P, D], F32, tag="sq")
                sumsq = pool.tile([P, 1], F32, tag="sumsq")
                nc.scalar.activation(sq[:], xt[:], Act.Square, accum_out=sumsq[:])
                msq = pool.tile([P, 1], F32, tag="msq")
                nc.vector.tensor_tensor(msq[:], mean[:], mean[:], op=Alu.mult)
                var = pool.tile([P, 1], F32, tag="var")
                nc.vector.tensor_scalar(var[:], sumsq[:], 1.0 / D, EPS, Alu.mult, Alu.add)
                nc.vector.tensor_tensor(var[:], var[:], msq[:], op=Alu.subtract)
                rinv = pool.tile([P, 1], F32, tag="rinv")
                nc.vector.reciprocal(rinv[:], var[:])
                rstd = pool.tile([P, 1], F32, tag="rstd")
                nc.scalar.sqrt(rstd[:], rinv[:])
                hh = pool.tile([P, D], F32, tag="hh")
                nc.vector.tensor_scalar(
                    hh[:], xt[:], mean[:], rstd[:], Alu.subtract, Alu.mult
                )
                for dblk in range(8):
                    pt = ps_pool.tile([P, P], F32, tag="lnt")
                    nc.tensor.transpose(pt[:], hh[:, dblk * P : (dblk + 1) * P], ident[:])
                    nc.scalar.activation(
                        dstT[:, dblk, t, :],
                        pt[:],
                        Act.Identity,
                        bias=b_sb[:, dblk : dblk + 1],
                        scale=w_sb[:, dblk : dblk + 1],
                    )

            with (
                tc.tile_pool(name="ph1", bufs=2) as ph1,
                tc.tile_pool(name="ph1T", bufs=1) as ph1T,
                tc.tile_pool(name="psA", bufs=2, space="PSUM") as psA,
                nc.named_scope("ph1_ln1"),
            ):
                hT_asm = ph1T.tile([P, 8, 8, P], HDT)  # [p, dblk, t, j]
                TPC = 8 // NCH  # token tiles per chunk
                for ci in range(NCH):
                    for t in range(TPC * ci, TPC * ci + TPC):
                        xt = ph1.tile([P, D], F32, tag="xt")
                        nc.sync.dma_start(xt[:], x_own[t * P : (t + 1) * P, :])
                        layernorm_tile(ph1, xt, ln1w_sb, ln1b_sb, psA, hT_asm, t)
                    hTv = hT_dram[ci].rearrange("(dblk p) t -> p dblk t", p=P)
                    for dblk in range(8):
                        nc.sync.dma_start(
                            hTv[:, dblk, :],
                            hT_asm[:, dblk, TPC * ci : TPC * ci + TPC, :],
                        )
                    nc.gpsimd.collective_compute(
                        "AllGather", Alu.bypass, replica_groups=groups,
                        ins=[hT_dram[ci][:]], outs=[hT_full[ci][:]],
                    )

            # weights for ph4/ph5, prefetched during the AllGather wait
            # window (DMA engines are idle there)
            wpre_cm = tc.tile_pool(name="wpre", bufs=1)
            wpre = wpre_cm.__enter__()
            wproj_sb = wpre.tile([P, 8, D], BF16)
            nc.sync.dma_start(
                wproj_sb[:], wproj.rearrange("(ko p) n -> p ko n", p=P)
            )
            wfc2_sb = wpre.tile([P, DFF // P, D], BF16)
            nc.sync.dma_start(
                wfc2_sb[:], wfc2.rearrange("(kb p) n -> p kb n", p=P)
            )

            # =========================================================
            # Phase 2: q^T, k^T (bf16, both heads stacked on 128
            # partitions) and v-natural (vn, with 64 ones-columns per
            # head) per gathered chunk.  Gathered chunk ci strip rr
            # holds batches {2ci, 2ci+1}: local t in [0,512) -> batch
            # 2ci + t//256, global tok 2048*(2ci+t//256) + 256*rr + t%256
            # =========================================================
            with tc.tile_pool(name="qkv", bufs=1) as qkvp:
                qT = qkvp.tile([P, 16, 512], BF16)
                kT = qkvp.tile([P, 16, 512], BF16)
                # vn[kv_p, g, h, 0:64] = v dims of head h for kv block g;
                # cols 64:128 = 1.0 (softmax-sum columns for the AV mm)
                vn = qkvp.tile([P, 64, 2, P], BF16)
                nc.vector.memset(vn[:, :, :, HD:P], 1.0)
                with (
                    tc.tile_pool(name="wqkv", bufs=1) as wp,
                    tc.tile_pool(name="ph2", bufs=3) as ph2,
                    tc.tile_pool(name="psB", bufs=3, space="PSUM") as psB,
                    tc.tile_pool(name="psBT", bufs=2, space="PSUM") as psBT,
                    nc.named_scope("ph2_qkv"),
                ):
                    wq_sb = wp.tile([P, 8, P], BF16)
                    nc.sync.dma_start(wq_sb[:], wq.rearrange("(ko p) m -> p ko m", p=P))
                    wk_sb = wp.tile([P, 8, P], BF16)
                    nc.sync.dma_start(wk_sb[:], wk.rearrange("(ko p) m -> p ko m", p=P))
                    wv_sb = wp.tile([P, 8, P], BF16)
                    nc.sync.dma_start(wv_sb[:], wv.rearrange("(ko p) m -> p ko m", p=P))
                    hfvs = [hq.rearrange("(r ko p) t -> r p ko t", p=P, ko=8)
                            for hq in hT_full]
                    for ci in range(NCH):
                        hfv = hfvs[ci]
                        for rr in range(8):
                            ht = ph2.tile([P, 8, CHT], HDT, tag="ht")
                            nc.sync.dma_start(ht[:], hfv[rr])
                            # two 256-token halves -> two tile16 slots
                            t16a = 8 * ci + rr // 2
                            co = (rr % 2) * STR
                            sls = [
                                (slice(None), t16a, slice(co, co + STR)),
                                (slice(None), t16a + 4, slice(co, co + STR)),
                            ]
                            for wi, (w_sb, b_sb, dstT) in enumerate(
                                ((wq_sb, bq_sb, qT), (wk_sb, bk_sb, kT),
                                 (wv_sb, bv_sb, None))
                            ):
                                ps = psB.tile([P, CHT], F32, tag="qkvps")
                                for ko in range(8):
                                    nc.tensor.matmul(
                                        ps[:], w_sb[:, ko, :], ht[:, ko, :],
                                        start=(ko == 0), stop=(ko == 7),
                                    )
                                if dstT is not None:
                                    for half in range(2):
                                        nc.scalar.activation(
                                            dstT[sls[half]],
                                            ps[:, half * STR : half * STR + STR],
                                            Act.Identity, bias=b_sb[:],
                                        )
                                else:
                                    # v: bias, transpose to natural, pack
                                    # into vn (dims on cols, split heads)
                                    vtmp = ph2.tile([P, CHT], BF16, tag="vtmp")
                                    nc.scalar.activation(
                                        vtmp[:], ps[:], Act.Identity,
                                        bias=b_sb[:],
                                    )
                                    for jp in range(2):  # pairs of kv blocks
                                        pt = psBT.tile([P, 2, P], BF16, tag="vt")
                                        for j2 in range(2):
                                            j = jp * 2 + j2
                                            nc.tensor.transpose(
                                                pt[:, j2, :],
                                                vtmp[:, j * P : (j + 1) * P],
                                                ident_bf[:],
                                            )
                                        g0 = 32 * ci + 16 * jp + 2 * rr
                                        nc.vector.tensor_copy(
                                            vn[:, g0 : g0 + 2, :, 0:HD],
                                            pt[:].rearrange(
                                                "p j (h c) -> p j h c", h=2),
                                        )

                # =====================================================
                # Phase 3: causal attention, batch-major; both heads
                # concurrently via row-tiled K=64 score matmuls.
                # =====================================================
                with (
                    tc.tile_pool(name="ph3", bufs=4) as ph3,
                    tc.tile_pool(name="ph3s", bufs=3) as ph3s,
                    tc.tile_pool(name="psQK", bufs=2, space="PSUM") as psQK,
                    tc.tile_pool(name="psY", bufs=2, space="PSUM") as psY,
                    nc.named_scope("ph3_attn"),
                ):
                    for b in range(B):
                        for tq in (3, 2, 1, 0):
                            nkv = 4 * (tq + 1)
                            py = psY.tile([P, 2, 512], F32, tag="py")
                            pend = []  # pipelined AV emission
                            for kb in range(nkv):
                                diag = kb >= 4 * tq
                                ps = psQK.tile([P, 2, 512], F32, tag="qk")
                                for h in range(2):
                                    hp = h * HD
                                    nc.tensor.matmul(
                                        ps[:, h, :],
                                        kT[hp : hp + HD, b * 4 + kb // 4,
                                           (kb % 4) * P : (kb % 4 + 1) * P],
                                        qT[hp : hp + HD, b * 4 + tq, :],
                                        start=True, stop=True,
                                    )
                                ex = ph3.tile([P, 2, 512], BF16, tag="ex")
                                nc.scalar.activation(
                                    ex[:], ps[:], Act.Exp, scale=1.0 / np.sqrt(HD)
                                )
                                if diag:  # zero the causal-invalid region
                                    s_off = kb - 4 * tq
                                    mw = P * (s_off + 1)
                                    nc.vector.tensor_tensor(
                                        ex[:, :, 0:mw], ex[:, :, 0:mw],
                                        mask01[:, s_off, :, 0:mw],
                                        op=Alu.mult,
                                    )
                                pend.append((kb, ex))
                                if len(pend) > 1:  # one score block ahead
                                    kb0, e0 = pend.pop(0)
                                    for h in range(2):
                                        nc.tensor.matmul(
                                            py[:, h, :],
                                            vn[:, 16 * b + kb0, h, :],
                                            e0[:, h, :],
                                            start=(kb0 == 0), stop=False,
                                        )
                            while pend:
                                kb0, e0 = pend.pop(0)
                                for h in range(2):
                                    nc.tensor.matmul(
                                        py[:, h, :],
                                        vn[:, 16 * b + kb0, h, :],
                                        e0[:, h, :],
                                        start=(kb0 == 0),
                                        stop=(not pend),
                                    )
                            # normalize: rows 64..127 hold the softmax
                            # sums (replicated by the ones columns)
                            for h in range(2):
                                yt = ph3s.tile([HD, 512], BF16, tag="yt")
                                if USE_DIVIDE:
                                    nc.vector.tensor_tensor(
                                        yt[:], py[0:HD, h, :], py[HD:P, h, :],
                                        op=Alu.divide,
                                    )
                                else:
                                    rec = ph3s.tile([HD, 512], F32, tag="rec")
                                    nc.vector.reciprocal(rec[:], py[HD:P, h, :])
                                    nc.vector.tensor_tensor(
                                        yt[:], py[0:HD, h, :], rec[:], op=Alu.mult
                                    )
                                # tq covers strips r = 2tq, 2tq+1
                                ysv = y_send[b].rearrange("r h p t -> h p r t")
                                nc.sync.dma_start(
                                    ysv[h, :, 2 * tq : 2 * tq + 2, :],
                                    yt[:].rearrange("p (r t) -> p r t", r=2),
                                )
                        with nc.named_scope(f"cc_a2a_y{b}"):
                            nc.gpsimd.collective_compute(
                                "AllToAll", Alu.bypass, replica_groups=groups,
                                ins=[y_send[b][:]], outs=[y_recv[b][:]],
                            )

            # =========================================================
            # Phase 4: proj + residual + LN2 (own tokens) -> h2^T (SBUF)
            # own t-tile t = batch t//2, strip off (t%2)*128
            # =========================================================
            with tc.tile_pool(name="keep", bufs=1) as keep:
                h2T = keep.tile([P, 8, 8, P], BF16)  # [p, dblk, t, j]
                x1_sb = keep.tile([P, 8, D], F32)    # [p, t, d]
                with (
                    tc.tile_pool(name="ph4", bufs=2) as ph4,
                    tc.tile_pool(name="psC", bufs=4, space="PSUM") as psC,
                    tc.tile_pool(name="psD", bufs=2, space="PSUM") as psD,
                    nc.named_scope("ph4_proj_ln2"),
                ):
                    # y_recv[b][src, h, p, t]: ydim = 128*src + 64*h + p
                    yrv = [y_recv[b].rearrange("s h p t -> h p s t")
                           for b in range(B)]
                    for t in range(8):
                        yv = yrv[t // 2]
                        off = (t % 2) * P
                        yt_own = ph4.tile([P, 8, P], BF16, tag="ytown")
                        nc.sync.dma_start(
                            yt_own[0:HD, :, :], yv[0][:, :, off : off + P]
                        )
                        nc.sync.dma_start(
                            yt_own[HD:P, :, :], yv[1][:, :, off : off + P]
                        )
                        xt = ph4.tile([P, D], F32, tag="xt4")
                        nc.sync.dma_start(xt[:], x_own[t * P : (t + 1) * P, :])
                        for n in range(2):
                            pp = psC.tile([P, 512], F32, tag="pj")
                            if use_bproj:
                                nc.tensor.matmul(
                                    pp[:], ones_b[:, :P],
                                    bproj_sb[:, n * 512 : (n + 1) * 512],
                                    start=True, stop=False,
                                )
                            for ko in range(8):
                                nc.tensor.matmul(
                                    pp[:], yt_own[:, ko, :],
                                    wproj_sb[:, ko, n * 512 : (n + 1) * 512],
                                    start=(ko == 0 and not use_bproj),
                                    stop=(ko == 7),
                                )
                            nc.vector.tensor_tensor(
                                x1_sb[:, t, n * 512 : (n + 1) * 512], pp[:],
                                xt[:, n * 512 : (n + 1) * 512], op=Alu.add,
                            )
                        layernorm_tile(
                            ph4, x1_sb[:, t, :], ln2w_sb, ln2b_sb, psD, h2T, t
                        )

                # =====================================================
                # Phase 5: MLP (own tokens, 2 groups of 512, bf16)
                # =====================================================
                with (
                    tc.tile_pool(name="mt", bufs=1) as mtp,
                    tc.tile_pool(name="ph5", bufs=3) as ph5,
                    tc.tile_pool(name="psM", bufs=2, space="PSUM") as psM,
                    tc.tile_pool(name="psO", bufs=1, space="PSUM") as psO,
                    nc.named_scope("ph5_mlp"),
                ):
                    wfcv = wfc.rearrange("(ko p) n -> p ko n", p=P)
                    for g in range(2):
                        mT = mtp.tile([P, DFF // P, 512], BF16, tag="mt")
                        for kb in range(DFF // P):
                            wt = ph5.tile([P, 8, P], BF16, tag="wfct")
                            nc.sync.dma_start(
                                wt[:], wfcv[:, :, kb * P : (kb + 1) * P]
                            )
                            pm = psM.tile([P, 512], F32, tag="pm")
                            for ko in range(8):
                                nc.tensor.matmul(
                                    pm[:], wt[:, ko, :],
                                    h2T[:, ko, 4 * g : 4 * g + 4, :],
                                    start=(ko == 0), stop=(ko == 7),
                                )
                            nc.scalar.activation(
                                mT[:, kb, :], pm[:], Act.Gelu,
                                bias=bfc_sb[:, kb : kb + 1],
                            )
                        for n2 in range(2):
                            pos = [
                                psO.tile([P, 512], F32, tag=f"po{t2}",
                                         name=f"po_{g}_{n2}_{t2}")
                                for t2 in range(4)
                            ]
                            if use_bfc2:
                                for t2 in range(4):
                                    nc.tensor.matmul(
                                        pos[t2][:], ones_b[:, :P],
                                        bfc2_sb[:, n2 * 512 : (n2 + 1) * 512],
                                        start=True, stop=False,
                                    )
                            for kb in range(DFF // P):
                                for t2 in range(4):
                                    nc.tensor.matmul(
                                        pos[t2][:], mT[:, kb, t2 * P : (t2 + 1) * P],
                                        wfc2_sb[:, kb, n2 * 512 : (n2 + 1) * 512],
                                        start=(kb == 0 and not use_bfc2),
                                        stop=(kb == DFF // P - 1),
                                    )
                            for t2 in range(4):
                                t = 4 * g + t2
                                ot = ph5.tile([P, 512], F32, tag="ot")
                                nc.vector.tensor_tensor(
                                    ot[:], pos[t2][:],
                                    x1_sb[:, t, n2 * 512 : (n2 + 1) * 512],
                                    op=Alu.add,
                                )
                                nc.sync.dma_start(
                                    out_own[t * P : (t + 1) * P,
                                            n2 * 512 : (n2 + 1) * 512],
                                    ot[:],
                                )
            wpre_cm.__exit__(None, None, None)

    nc.compile()
    return nc


_NC_CACHE = {}


def kernel(x, ln1_w, ln1_b, ln2_w, ln2_b, w_attn, b_attn, w_proj, b_proj,
           w_fc, b_fc, w_fc2, b_fc2):
    f = np.ascontiguousarray
    x = np.asarray(x, np.float32)
    w_attn = np.asarray(w_attn, np.float32)
    b_attn = np.asarray(b_attn, np.float32)
    b_proj = np.asarray(b_proj, np.float32)
    b_fc2 = np.asarray(b_fc2, np.float32)

    key = (bool(np.any(b_proj)), bool(np.any(b_fc2)))
    if key not in _NC_CACHE:
        _NC_CACHE[key] = build(use_bproj=key[0], use_bfc2=key[1])
    nc = _NC_CACHE[key]
    global _NC_LAST
    _NC_LAST = nc

    # striped ownership: core c owns strip c of every batch
    xs = x.reshape(B, NCORES, STR, D)  # [b, strip, 256, d]

    def col(v, c):  # [128, 1] bias slice
        return f(np.asarray(v, np.float32)[c * P : (c + 1) * P].reshape(P, 1))

    def strip(v):  # [1024] -> [128, 8] with [p, a] = v[a*128 + p]
        return f(np.asarray(v, np.float32).reshape(-1, P).T)

    def bf(v):
        return f(np.asarray(v, np.float32).astype(BF))

    in_maps = []
    for c in range(NCORES):
        in_maps.append({
            "x_own": f(xs[:, c].reshape(TOWN, D)),
            "wq": bf(w_attn[:, P * c : P * (c + 1)]),
            "wk": bf(w_attn[:, D + P * c : D + P * (c + 1)]),
            "wv": bf(w_attn[:, 2 * D + P * c : 2 * D + P * (c + 1)]),
            "bq": col(b_attn, c),
            "bk": col(b_attn[D:], c),
            "bv": col(b_attn[2 * D:], c),
            "ln1w": strip(ln1_w), "ln1b": strip(ln1_b),
            "ln2w": strip(ln2_w), "ln2b": strip(ln2_b),
            "wproj": bf(w_proj),
            "bproj": bf(b_proj.reshape(1, D)),
            "wfc": bf(w_fc),
            "bfc": strip(b_fc),
            "wfc2": bf(w_fc2),
            "bfc2": bf(b_fc2.reshape(1, D)),
        })

    global _last_in_maps
    _last_in_maps = in_maps
    res = run_bass_kernel_spmd(nc, in_maps, core_ids=list(range(NCORES)))
    # reassemble: core c's rows are [b, strip c] pieces
    out = np.empty((B, NCORES, STR, D), np.float32)
    for c in range(NCORES):
        out[:, c] = res.results[c]["out"].reshape(B, STR, D)
    return out.reshape(B, T, D)


_NC_LAST = None
_last_in_maps = None


# revision 19
# speedup vs baseline: 1.2463x; 1.0592x over previous
"""Trainium2 Bass kernel for a GPT-style transformer block.

Reference computation (B=4, T=2048, d=1024, 16 heads, dff=4096, fp32):
    h  = LN1(x);  qkv = h @ w_attn + b_attn
    y  = causal_attention(q, k, v);  x1 = x + y @ w_proj + b_proj
    h2 = LN2(x1); out = x1 + gelu(h2 @ w_fc + b_fc) @ w_fc2 + b_fc2

Sharding over 8 NeuronCores (one trn2 chip), STRIPED token ownership:
  core c owns token strip [2048*b + 256*c, +256) of every batch b (1024
  tokens total).  This alignment makes each LN1 AllGather chunk ci
  deliver exactly batch ci's tokens, and lets the per-head attention
  output redistribute with one small AllToAll per batch, pipelined
  behind the next batch's attention.

  - ph1: LN1 over own tokens in 4 chunks of 256; each chunk's h^T shard
    (bf16) AllGathers as soon as it is ready.  A 1-byte dummy AllGather
    at kernel start absorbs the ~50us collective-init barrier.
  - ph2: per gathered chunk, q^T/k^T/v^T (bf16, both heads stacked
    on 128 partitions) for this core's 2 heads over that batch.
  - ph3: causal attention, batch-major.  Scores for the two heads run
    CONCURRENTLY as row-tiled K=64 matmuls (rows 0-63 / 64-127 of the
    PE array) into separate PSUM banks.  exp on ACT; causal masking by
    a 0/1 bf16 multiply on DVE (diagonal blocks only); softmax
    denominator comes from 64 ones-columns appended to v, and the
    normalization reciprocal is computed as exp(-ln(s)) on ACT (DVE
    reciprocal is ~16x slower).  After each batch, one AllToAll
    redistributes y to token owners, overlapped with the next batch.
  - ph4/ph5: token-parallel proj+residual+LN2 and MLP with full-width
    weights, as in the reference.

Matmul-shape notes: matmul time = moving-free-dim cycles (independent
of K and M), so the 64-ones columns and K=64 padding are free; what
matters is slot count, which row-tiling halves for scores.  Bias
matmuls (ones-row trick) are skipped at build time when the bias
vectors are all zero (they are, for this problem's inputs).
"""

import sys

import numpy as np
import ml_dtypes

sys.path.insert(0, "/opt/trn_rl_repo")

import concourse.bass as bass  # noqa: E402
import concourse.mybir as mybir  # noqa: E402
import concourse.tile as tile  # noqa: E402
from concourse import bacc  # noqa: E402
from concourse.bass_utils import run_bass_kernel_spmd  # noqa: E402
from concourse.masks import make_identity  # noqa: E402

B, T, D, H, HD, DFF = 4, 2048, 1024, 16, 64, 4096
EPS = 1e-5
NCORES = 8
TOK = B * T            # 8192 flattened tokens
TOWN = TOK // NCORES   # 1024 tokens owned per core
STR = 256              # per-batch strip owned per core
P = 128
F32 = mybir.dt.float32
BF16 = mybir.dt.bfloat16
FP8 = mybir.dt.float8e4
Act = mybir.ActivationFunctionType
Alu = mybir.AluOpType
AX = mybir.AxisListType
BF = ml_dtypes.bfloat16

H_FP8 = True        # gather h in fp8e4m3 (halves the AllGather bytes)
USE_DIVIDE = False  # DVE tensor_tensor divide fails neuronxcc codegen
HDT = FP8 if H_FP8 else BF16


def build(use_bproj=True, use_bfc2=True):
    nc = bacc.Bacc("TRN2", target_bir_lowering=False, debug=False, num_devices=NCORES)

    def inp(name, shape, dt=F32):
        return nc.dram_tensor(name, shape, dt, kind="ExternalInput").ap()

    x_own = inp("x_own", [TOWN, D])
    wq = inp("wq", [D, P], BF16)
    wk = inp("wk", [D, P], BF16)
    wv = inp("wv", [D, P], BF16)
    bq = inp("bq", [P, 1])
    bk = inp("bk", [P, 1])
    bv = inp("bv", [P, 1])
    ln1w = inp("ln1w", [P, 8])
    ln1b = inp("ln1b", [P, 8])
    ln2w = inp("ln2w", [P, 8])
    ln2b = inp("ln2b", [P, 8])
    wproj = inp("wproj", [D, D], BF16)
    bproj = inp("bproj", [1, D], BF16)
    wfc = inp("wfc", [D, DFF], BF16)
    bfc = inp("bfc", [P, DFF // P])
    wfc2 = inp("wfc2", [DFF, D], BF16)
    bfc2 = inp("bfc2", [1, D], BF16)
    out_own = nc.dram_tensor("out", [TOWN, D], F32, kind="ExternalOutput").ap()

    groups = [list(range(NCORES))]

    with tile.TileContext(nc) as tc:
        with (
            tc.tile_pool(name="const", bufs=1) as cst,
            tc.tile_pool(name="dram", bufs=1, space="DRAM") as dram,
        ):
            # ---------------- constants ----------------
            ident = cst.tile([P, P], F32)
            make_identity(nc, ident)
            ident_bf = cst.tile([P, P], BF16)
            make_identity(nc, ident_bf)
            ones_b = None
            if use_bproj or use_bfc2:
                ones_f = cst.tile([1, P], F32)
                nc.vector.memset(ones_f[:], 1.0)
                ones_b = cst.tile([1, P], BF16)
                nc.scalar.copy(ones_b[:], ones_f[:])
            ln1w_sb = cst.tile([P, 8], F32)
            nc.sync.dma_start(ln1w_sb[:], ln1w)
            ln1b_sb = cst.tile([P, 8], F32)
            nc.sync.dma_start(ln1b_sb[:], ln1b)
            ln2w_sb = cst.tile([P, 8], F32)
            nc.sync.dma_start(ln2w_sb[:], ln2w)
            ln2b_sb = cst.tile([P, 8], F32)
            nc.sync.dma_start(ln2b_sb[:], ln2b)
            bq_sb = cst.tile([P, 1], F32)
            nc.sync.dma_start(bq_sb[:], bq)
            bk_sb = cst.tile([P, 1], F32)
            nc.sync.dma_start(bk_sb[:], bk)
            bv_sb = cst.tile([P, 1], F32)
            nc.sync.dma_start(bv_sb[:], bv)
            bproj_sb = None
            if use_bproj:
                bproj_sb = cst.tile([1, D], BF16)
                nc.sync.dma_start(bproj_sb[:], bproj)
            bfc_sb = cst.tile([P, DFF // P], F32)
            nc.sync.dma_start(bfc_sb[:], bfc)
            bfc2_sb = None
            if use_bfc2:
                bfc2_sb = cst.tile([1, D], BF16)
                nc.sync.dma_start(bfc2_sb[:], bfc2)
            # 0/1 causal masks for the 4 diagonal offsets, replicated for
            # both heads: mask01[s][i, h, j] = 1 if i <= j - 128*s else 0
            mask01 = cst.tile([P, 4, 2, 512], BF16)
            nc.vector.memset(mask01[:], 1.0)
            for s in range(4):
                for h in range(2):
                    nc.gpsimd.affine_select(
                        out=mask01[:, s, h, :],
                        in_=mask01[:, s, h, :],
                        pattern=[[1, 512]],
                        channel_multiplier=-1,
                        base=-128 * s,
                        compare_op=Alu.is_ge,
                        fill=0.0,
                    )

            # DRAM intermediates.
            # hT chunk ci: own tokens [512ci, 512ci+512) -> gathered chunk
            # ci holds batches {2ci, 2ci+1} as [8 strips, ...].
            NCH = 2
            CHT = TOWN // NCH  # own tokens per chunk
            hT_dram = [dram.tile([D, CHT], HDT, name=f"hq{i}") for i in range(NCH)]
            hT_full = [dram.tile([NCORES * D, CHT], HDT, addr_space="Shared",
                                 name=f"hfq{i}") for i in range(NCH)]
            # per-batch y AllToAll: slice r = my 2 heads' y for core r's
            # strip of this batch.
            y_send = [dram.tile([NCORES, 2, HD, STR], BF16, name=f"ys{b}")
                      for b in range(B)]
            y_recv = [dram.tile([NCORES, 2, HD, STR], BF16, name=f"yr{b}")
                      for b in range(B)]

            # =========================================================
            # Phase 1: LN1 over own tokens, 4 chunks -> AllGather each
            # =========================================================
            def layernorm_tile(pool, xt, w_sb, b_sb, ps_pool, dstT, t):
                """LN a [128, D] token tile and write transposed blocks
                (with gamma/beta applied) into dstT[:, dblk, t, :] (bf16)."""
                ssum = pool.tile([P, 1], F32, tag="ssum")
                nc.vector.reduce_sum(ssum[:], xt[:], axis=AX.X)
                mean = pool.tile([P, 1], F32, tag="mean")
                nc.scalar.mul(mean[:], ssum[:], 1.0 / D)
                sq = pool.tile([P, D], F32, tag="sq")
                sumsq = pool.tile([P, 1], F32, tag="sumsq")
                nc.scalar.activation(sq[:], xt[:], Act.Square, accum_out=sumsq[:])
                msq = pool.tile([P, 1], F32, tag="msq")
                nc.vector.tensor_tensor(msq[:], mean[:], mean[:], op=Alu.mult)
                var = pool.tile([P, 1], F32, tag="var")
                nc.vector.tensor_scalar(var[:], sumsq[:], 1.0 / D, EPS, Alu.mult, Alu.add)
                nc.vector.tensor_tensor(var[:], var[:], msq[:], op=Alu.subtract)
                rinv = pool.tile([P, 1], F32, tag="rinv")
                nc.vector.reciprocal(rinv[:], var[:])
                rstd = pool.tile([P, 1], F32, tag="rstd")
                nc.scalar.sqrt(rstd[:], rinv[:])
                hh = pool.tile([P, D], F32, tag="hh")
                nc.vector.tensor_scalar(
                    hh[:], xt[:], mean[:], rstd[:], Alu.subtract, Alu.mult
                )
                for dblk in range(8):
                    pt = ps_pool.tile([P, P], F32, tag="lnt")
                    nc.tensor.transpose(pt[:], hh[:, dblk * P : (dblk + 1) * P], ident[:])
                    nc.scalar.activation(
                        dstT[:, dblk, t, :],
                        pt[:],
                        Act.Identity,
                        bias=b_sb[:, dblk : dblk + 1],
                        scale=w_sb[:, dblk : dblk + 1],
                    )

            with (
                tc.tile_pool(name="ph1", bufs=2) as ph1,
                tc.tile_pool(name="ph1T", bufs=1) as ph1T,
                tc.tile_pool(name="psA", bufs=2, space="PSUM") as psA,
                nc.named_scope("ph1_ln1"),
            ):
                hT_asm = ph1T.tile([P, 8, 8, P], HDT)  # [p, dblk, t, j]
                TPC = 8 // NCH  # token tiles per chunk
                for ci in range(NCH):
                    for t in range(TPC * ci, TPC * ci + TPC):
                        xt = ph1.tile([P, D], F32, tag="xt")
                        nc.sync.dma_start(xt[:], x_own[t * P : (t + 1) * P, :])
                        layernorm_tile(ph1, xt, ln1w_sb, ln1b_sb, psA, hT_asm, t)
                    hTv = hT_dram[ci].rearrange("(dblk p) t -> p dblk t", p=P)
                    for dblk in range(8):
                        nc.sync.dma_start(
                            hTv[:, dblk, :],
                            hT_asm[:, dblk, TPC * ci : TPC * ci + TPC, :],
                        )
                    nc.gpsimd.collective_compute(
                        "AllGather", Alu.bypass, replica_groups=groups,
                        ins=[hT_dram[ci][:]], outs=[hT_full[ci][:]],
                    )

            # weights for ph4/ph5, prefetched during the AllGather wait
            # window (DMA engines are idle there)
            wpre_cm = tc.tile_pool(name="wpre", bufs=1)
            wpre = wpre_cm.__enter__()
            wproj_sb = wpre.tile([P, 8, D], BF16)
            nc.sync.dma_start(
                wproj_sb[:], wproj.rearrange("(ko p) n -> p ko n", p=P)
            )
            wfc2_sb = wpre.tile([P, DFF // P, D], BF16)
            nc.sync.dma_start(
                wfc2_sb[:], wfc2.rearrange("(kb p) n -> p kb n", p=P)
            )

            # =========================================================
            # Phase 2: q^T, k^T (bf16, both heads stacked on 128
            # partitions) and v-natural (vn, with 64 ones-columns per
            # head) per gathered chunk.  Gathered chunk ci strip rr
            # holds batches {2ci, 2ci+1}: local t in [0,512) -> batch
            # 2ci + t//256, global tok 2048*(2ci+t//256) + 256*rr + t%256
            # =========================================================
            with tc.tile_pool(name="qkv", bufs=1) as qkvp:
                qT = qkvp.tile([P, 16, 512], BF16)
                kT = qkvp.tile([P, 16, 512], BF16)
                # vn[kv_p, g, h, 0:64] = v dims of head h for kv block g;
                # cols 64:128 = 1.0 (softmax-sum columns for the AV mm)
                vn = qkvp.tile([P, 64, 2, P], BF16)
                nc.vector.memset(vn[:, :, :, HD:P], 1.0)
                with (
                    tc.tile_pool(name="wqkv", bufs=1) as wp,
                    tc.tile_pool(name="ph2", bufs=3) as ph2,
                    tc.tile_pool(name="psB", bufs=3, space="PSUM") as psB,
                    tc.tile_pool(name="psBT", bufs=2, space="PSUM") as psBT,
                    nc.named_scope("ph2_qkv"),
                ):
                    wq_sb = wp.tile([P, 8, P], BF16)
                    nc.sync.dma_start(wq_sb[:], wq.rearrange("(ko p) m -> p ko m", p=P))
                    wk_sb = wp.tile([P, 8, P], BF16)
                    nc.sync.dma_start(wk_sb[:], wk.rearrange("(ko p) m -> p ko m", p=P))
                    wv_sb = wp.tile([P, 8, P], BF16)
                    nc.sync.dma_start(wv_sb[:], wv.rearrange("(ko p) m -> p ko m", p=P))
                    hfvs = [hq.rearrange("(r ko p) t -> r p ko t", p=P, ko=8)
                            for hq in hT_full]
                    for ci in range(NCH):
                        hfv = hfvs[ci]
                        for rr in range(8):
                            ht = ph2.tile([P, 8, CHT], HDT, tag="ht")
                            nc.sync.dma_start(ht[:], hfv[rr])
                            # two 256-token halves -> two tile16 slots
                            t16a = 8 * ci + rr // 2
                            co = (rr % 2) * STR
                            sls = [
                                (slice(None), t16a, slice(co, co + STR)),
                                (slice(None), t16a + 4, slice(co, co + STR)),
                            ]
                            for wi, (w_sb, b_sb, dstT) in enumerate(
                                ((wq_sb, bq_sb, qT), (wk_sb, bk_sb, kT),
                                 (wv_sb, bv_sb, None))
                            ):
                                ps = psB.tile([P, CHT], F32, tag="qkvps")
                                for ko in range(8):
                                    nc.tensor.matmul(
                                        ps[:], w_sb[:, ko, :], ht[:, ko, :],
                                        start=(ko == 0), stop=(ko == 7),
                                    )
                                if dstT is not None:
                                    for half in range(2):
                                        nc.scalar.activation(
                                            dstT[sls[half]],
                                            ps[:, half * STR : half * STR + STR],
                                            Act.Identity, bias=b_sb[:],
                                        )
                                else:
                                    # v: bias, transpose to natural, pack
                                    # into vn (dims on cols, split heads)
                                    vtmp = ph2.tile([P, CHT], BF16, tag="vtmp")
                                    nc.scalar.activation(
                                        vtmp[:], ps[:], Act.Identity,
                                        bias=b_sb[:],
                                    )
                                    for jp in range(2):  # pairs of kv blocks
                                        pt = psBT.tile([P, 2, P], BF16, tag="vt")
                                        for j2 in range(2):
                                            j = jp * 2 + j2
                                            nc.tensor.transpose(
                                                pt[:, j2, :],
                                                vtmp[:, j * P : (j + 1) * P],
                                                ident_bf[:],
                                            )
                                        g0 = 32 * ci + 16 * jp + 2 * rr
                                        nc.vector.tensor_copy(
                                            vn[:, g0 : g0 + 2, :, 0:HD],
                                            pt[:].rearrange(
                                                "p j (h c) -> p j h c", h=2),
                                        )

                # =====================================================
                # Phase 3: causal attention, batch-major; both heads
                # concurrently via row-tiled K=64 score matmuls.
                # =====================================================
                with (
                    tc.tile_pool(name="ph3", bufs=4) as ph3,
                    tc.tile_pool(name="ph3s", bufs=3) as ph3s,
                    tc.tile_pool(name="psQK", bufs=2, space="PSUM") as psQK,
                    tc.tile_pool(name="psY", bufs=2, space="PSUM") as psY,
                    nc.named_scope("ph3_attn"),
                ):
                    for b in range(B):
                        for tq in (3, 2, 1, 0):
                            nkv = 4 * (tq + 1)
                            py = psY.tile([P, 2, 512], F32, tag="py")
                            pend = []  # pipelined AV emission
                            for kb in range(nkv):
                                diag = kb >= 4 * tq
                                ps = psQK.tile([P, 2, 512], F32, tag="qk")
                                for h in range(2):
                                    hp = h * HD
                                    nc.tensor.matmul(
                                        ps[:, h, :],
                                        kT[hp : hp + HD, b * 4 + kb // 4,
                                           (kb % 4) * P : (kb % 4 + 1) * P],
                                        qT[hp : hp + HD, b * 4 + tq, :],
                                        start=True, stop=True,
                                    )
                                # diag block s: cols [0,128s) are fully
                                # causal-invalid -> memset 0, skip the exp
                                # there; cols [128s,128(s+1)) get the 0/1
                                # triangle mask; cols >= 128(s+1) are valid.
                                s_off = kb - 4 * tq
                                lo = P * s_off if s_off > 0 else 0
                                ex = ph3.tile([P, 2, 512], BF16, tag="ex")
                                if lo:
                                    nc.vector.memset(ex[:, :, 0:lo], 0.0)
                                nc.scalar.activation(
                                    ex[:, :, lo:], ps[:, :, lo:], Act.Exp,
                                    scale=1.0 / np.sqrt(HD)
                                )
                                if diag:
                                    mw = P * (s_off + 1)
                                    nc.vector.tensor_tensor(
                                        ex[:, :, lo:mw], ex[:, :, lo:mw],
                                        mask01[:, s_off, :, lo:mw],
                                        op=Alu.mult,
                                    )
                                pend.append((kb, ex))
                                if len(pend) > 1:  # one score block ahead
                                    kb0, e0 = pend.pop(0)
                                    for h in range(2):
                                        nc.tensor.matmul(
                                            py[:, h, :],
                                            vn[:, 16 * b + kb0, h, :],
                                            e0[:, h, :],
                                            start=(kb0 == 0), stop=False,
                                        )
                            while pend:
                                kb0, e0 = pend.pop(0)
                                for h in range(2):
                                    nc.tensor.matmul(
                                        py[:, h, :],
                                        vn[:, 16 * b + kb0, h, :],
                                        e0[:, h, :],
                                        start=(kb0 == 0),
                                        stop=(not pend),
                                    )
                            # normalize: rows 64..127 hold the softmax
                            # sums (replicated by the ones columns)
                            for h in range(2):
                                yt = ph3s.tile([HD, 512], BF16, tag="yt")
                                if USE_DIVIDE:
                                    nc.vector.tensor_tensor(
                                        yt[:], py[0:HD, h, :], py[HD:P, h, :],
                                        op=Alu.divide,
                                    )
                                else:
                                    # ~18-bit fast reciprocal: sums are in
                                    # [~0.05, ~5e3], far from the undefined
                                    # edge cases (0/denorm/inf)
                                    rec = ph3s.tile([HD, 512], F32, tag="rec")
                                    nc.vector.reciprocal_approx_fast(
                                        rec[:], py[HD:P, h, :])
                                    nc.vector.tensor_tensor(
                                        yt[:], py[0:HD, h, :], rec[:], op=Alu.mult
                                    )
                                # tq covers strips r = 2tq, 2tq+1
                                ysv = y_send[b].rearrange("r h p t -> h p r t")
                                nc.sync.dma_start(
                                    ysv[h, :, 2 * tq : 2 * tq + 2, :],
                                    yt[:].rearrange("p (r t) -> p r t", r=2),
                                )
                        with nc.named_scope(f"cc_a2a_y{b}"):
                            nc.gpsimd.collective_compute(
                                "AllToAll", Alu.bypass, replica_groups=groups,
                                ins=[y_send[b][:]], outs=[y_recv[b][:]],
                            )

            # =========================================================
            # Phase 4: proj + residual + LN2 (own tokens) -> h2^T (SBUF)
            # own t-tile t = batch t//2, strip off (t%2)*128
            # =========================================================
            with tc.tile_pool(name="keep", bufs=1) as keep:
                h2T = keep.tile([P, 8, 8, P], BF16)  # [p, dblk, t, j]
                x1_sb = keep.tile([P, 8, D], F32)    # [p, t, d]
                with (
                    tc.tile_pool(name="ph4", bufs=2) as ph4,
                    tc.tile_pool(name="psC", bufs=4, space="PSUM") as psC,
                    tc.tile_pool(name="psD", bufs=2, space="PSUM") as psD,
                    nc.named_scope("ph4_proj_ln2"),
                ):
                    # y_recv[b][src, h, p, t]: ydim = 128*src + 64*h + p
                    yrv = [y_recv[b].rearrange("s h p t -> h p s t")
                           for b in range(B)]
                    for t in range(8):
                        yv = yrv[t // 2]
                        off = (t % 2) * P
                        yt_own = ph4.tile([P, 8, P], BF16, tag="ytown")
                        nc.sync.dma_start(
                            yt_own[0:HD, :, :], yv[0][:, :, off : off + P]
                        )
                        nc.sync.dma_start(
                            yt_own[HD:P, :, :], yv[1][:, :, off : off + P]
                        )
                        xt = ph4.tile([P, D], F32, tag="xt4")
                        nc.sync.dma_start(xt[:], x_own[t * P : (t + 1) * P, :])
                        for n in range(2):
                            pp = psC.tile([P, 512], F32, tag="pj")
                            if use_bproj:
                                nc.tensor.matmul(
                                    pp[:], ones_b[:, :P],
                                    bproj_sb[:, n * 512 : (n + 1) * 512],
                                    start=True, stop=False,
                                )
                            for ko in range(8):
                                nc.tensor.matmul(
                                    pp[:], yt_own[:, ko, :],
                                    wproj_sb[:, ko, n * 512 : (n + 1) * 512],
                                    start=(ko == 0 and not use_bproj),
                                    stop=(ko == 7),
                                )
                            nc.vector.tensor_tensor(
                                x1_sb[:, t, n * 512 : (n + 1) * 512], pp[:],
                                xt[:, n * 512 : (n + 1) * 512], op=Alu.add,
                            )
                        layernorm_tile(
                            ph4, x1_sb[:, t, :], ln2w_sb, ln2b_sb, psD, h2T, t
                        )

                # =====================================================
                # Phase 5: MLP (own tokens, 2 groups of 512, bf16)
                # =====================================================
                with (
                    tc.tile_pool(name="mt", bufs=1) as mtp,
                    tc.tile_pool(name="ph5", bufs=3) as ph5,
                    tc.tile_pool(name="psM", bufs=2, space="PSUM") as psM,
                    tc.tile_pool(name="psO", bufs=1, space="PSUM") as psO,
                    nc.named_scope("ph5_mlp"),
                ):
                    wfcv = wfc.rearrange("(ko p) n -> p ko n", p=P)
                    for g in range(2):
                        mT = mtp.tile([P, DFF // P, 512], BF16, tag="mt")
                        for kb in range(DFF // P):
                            wt = ph5.tile([P, 8, P], BF16, tag="wfct")
                            nc.sync.dma_start(
                                wt[:], wfcv[:, :, kb * P : (kb + 1) * P]
                            )
                            pm = psM.tile([P, 512], F32, tag="pm")
                            for ko in range(8):
                                nc.tensor.matmul(
                                    pm[:], wt[:, ko, :],
                                    h2T[:, ko, 4 * g : 4 * g + 4, :],
                                    start=(ko == 0), stop=(ko == 7),
                                )
                            nc.scalar.activation(
                                mT[:, kb, :], pm[:], Act.Gelu,
                                bias=bfc_sb[:, kb : kb + 1],
                            )
                        for n2 in range(2):
                            pos = [
                                psO.tile([P, 512], F32, tag=f"po{t2}",
                                         name=f"po_{g}_{n2}_{t2}")
                                for t2 in range(4)
                            ]
                            if use_bfc2:
                                for t2 in range(4):
                                    nc.tensor.matmul(
                                        pos[t2][:], ones_b[:, :P],
                                        bfc2_sb[:, n2 * 512 : (n2 + 1) * 512],
                                        start=True, stop=False,
                                    )
                            for kb in range(DFF // P):
                                for t2 in range(4):
                                    nc.tensor.matmul(
                                        pos[t2][:], mT[:, kb, t2 * P : (t2 + 1) * P],
                                        wfc2_sb[:, kb, n2 * 512 : (n2 + 1) * 512],
                                        start=(kb == 0 and not use_bfc2),
                                        stop=(kb == DFF // P - 1),
                                    )
                            for t2 in range(4):
                                t = 4 * g + t2
                                ot = ph5.tile([P, 512], F32, tag="ot")
                                nc.vector.tensor_tensor(
                                    ot[:], pos[t2][:],
                                    x1_sb[:, t, n2 * 512 : (n2 + 1) * 512],
                                    op=Alu.add,
                                )
                                nc.sync.dma_start(
                                    out_own[t * P : (t + 1) * P,
                                            n2 * 512 : (n2 + 1) * 512],
                                    ot[:],
                                )
            wpre_cm.__exit__(None, None, None)

    nc.compile()
    return nc


_NC_CACHE = {}


def kernel(x, ln1_w, ln1_b, ln2_w, ln2_b, w_attn, b_attn, w_proj, b_proj,
           w_fc, b_fc, w_fc2, b_fc2):
    f = np.ascontiguousarray
    x = np.asarray(x, np.float32)
    w_attn = np.asarray(w_attn, np.float32)
    b_attn = np.asarray(b_attn, np.float32)
    b_proj = np.asarray(b_proj, np.float32)
    b_fc2 = np.asarray(b_fc2, np.float32)

    key = (bool(np.any(b_proj)), bool(np.any(b_fc2)))
    if key not in _NC_CACHE:
        _NC_CACHE[key] = build(use_bproj=key[0], use_bfc2=key[1])
    nc = _NC_CACHE[key]
    global _NC_LAST
    _NC_LAST = nc

    # striped ownership: core c owns strip c of every batch
    xs = x.reshape(B, NCORES, STR, D)  # [b, strip, 256, d]

    def col(v, c):  # [128, 1] bias slice
        return f(np.asarray(v, np.float32)[c * P : (c + 1) * P].reshape(P, 1))

    def strip(v):  # [1024] -> [128, 8] with [p, a] = v[a*128 + p]
        return f(np.asarray(v, np.float32).reshape(-1, P).T)

    def bf(v):
        return f(np.asarray(v, np.float32).astype(BF))

    in_maps = []
    for c in range(NCORES):
        in_maps.append({
            "x_own": f(xs[:, c].reshape(TOWN, D)),
            "wq": bf(w_attn[:, P * c : P * (c + 1)]),
            "wk": bf(w_attn[:, D + P * c : D + P * (c + 1)]),
            "wv": bf(w_attn[:, 2 * D + P * c : 2 * D + P * (c + 1)]),
            "bq": col(b_attn, c),
            "bk": col(b_attn[D:], c),
            "bv": col(b_attn[2 * D:], c),
            "ln1w": strip(ln1_w), "ln1b": strip(ln1_b),
            "ln2w": strip(ln2_w), "ln2b": strip(ln2_b),
            "wproj": bf(w_proj),
            "bproj": bf(b_proj.reshape(1, D)),
            "wfc": bf(w_fc),
            "bfc": strip(b_fc),
            "wfc2": bf(w_fc2),
            "bfc2": bf(b_fc2.reshape(1, D)),
        })

    global _last_in_maps
    _last_in_maps = in_maps
    res = run_bass_kernel_spmd(nc, in_maps, core_ids=list(range(NCORES)))
    # reassemble: core c's rows are [b, strip c] pieces
    out = np.empty((B, NCORES, STR, D), np.float32)
    for c in range(NCORES):
        out[:, c] = res.results[c]["out"].reshape(B, STR, D)
    return out.reshape(B, T, D)


_NC_LAST = None
_last_in_maps = None


# revision 26
# speedup vs baseline: 1.3249x; 1.0630x over previous
"""Trainium2 Bass kernel for a GPT-style transformer block.

Reference computation (B=4, T=2048, d=1024, 16 heads, dff=4096, fp32):
    h  = LN1(x);  qkv = h @ w_attn + b_attn
    y  = causal_attention(q, k, v);  x1 = x + y @ w_proj + b_proj
    h2 = LN2(x1); out = x1 + gelu(h2 @ w_fc + b_fc) @ w_fc2 + b_fc2

Sharding over 8 NeuronCores (one trn2 chip), STRIPED token ownership:
  core c owns token strip [2048*b + 256*c, +256) of every batch b (1024
  tokens total).  This alignment makes each LN1 AllGather chunk ci
  deliver exactly batch ci's tokens, and lets the per-head attention
  output redistribute with one small AllToAll per batch, pipelined
  behind the next batch's attention.

  - ph1: LN1 over own tokens in 4 chunks of 256; each chunk's h^T shard
    (bf16) AllGathers as soon as it is ready.  A 1-byte dummy AllGather
    at kernel start absorbs the ~50us collective-init barrier.
  - ph2: per gathered chunk, q^T/k^T/v^T (bf16, both heads stacked
    on 128 partitions) for this core's 2 heads over that batch.
  - ph3: causal attention, batch-major.  Scores for the two heads run
    CONCURRENTLY as row-tiled K=64 matmuls (rows 0-63 / 64-127 of the
    PE array) into separate PSUM banks.  exp on ACT; causal masking by
    a 0/1 bf16 multiply on DVE (diagonal blocks only); softmax
    denominator comes from 64 ones-columns appended to v, and the
    normalization reciprocal is computed as exp(-ln(s)) on ACT (DVE
    reciprocal is ~16x slower).  After each batch, one AllToAll
    redistributes y to token owners, overlapped with the next batch.
  - ph4/ph5: token-parallel proj+residual+LN2 and MLP with full-width
    weights, as in the reference.

Matmul-shape notes: matmul time = moving-free-dim cycles (independent
of K and M), so the 64-ones columns and K=64 padding are free; what
matters is slot count, which row-tiling halves for scores.  Bias
matmuls (ones-row trick) are skipped at build time when the bias
vectors are all zero (they are, for this problem's inputs).
"""

import sys

import numpy as np
import ml_dtypes

sys.path.insert(0, "/opt/trn_rl_repo")

import concourse.bass as bass  # noqa: E402
import concourse.mybir as mybir  # noqa: E402
import concourse.tile as tile  # noqa: E402
from concourse import bacc  # noqa: E402
from concourse.bass_utils import run_bass_kernel_spmd  # noqa: E402
from concourse.masks import make_identity  # noqa: E402

B, T, D, H, HD, DFF = 4, 2048, 1024, 16, 64, 4096
EPS = 1e-5
NCORES = 8
TOK = B * T            # 8192 flattened tokens
TOWN = TOK // NCORES   # 1024 tokens owned per core
STR = 256              # per-batch strip owned per core
P = 128
F32 = mybir.dt.float32
BF16 = mybir.dt.bfloat16
FP8 = mybir.dt.float8e4
Act = mybir.ActivationFunctionType
Alu = mybir.AluOpType
AX = mybir.AxisListType
BF = ml_dtypes.bfloat16

H_FP8 = True        # gather h in fp8e4m3 (halves the AllGather bytes)
FC1_DR = False      # fc1 fp8 fails the 2e-2 gate (gelu+fc2 amplify noise)
FC2_DR = True       # fc2 in fp8e4m3 with DoubleRow (K=256 per pass)
USE_DIVIDE = False  # DVE tensor_tensor divide fails neuronxcc codegen
FAST_RECIP = True   # reciprocal_approx_fast for softmax denominators
DIAG_SKIP = True    # skip exp on fully-invalid diag columns
HDT = FP8 if H_FP8 else BF16
WSC = 16.0  # fp8 weight pre-scale (w*0.02 sigma is denormal in e4m3)
DR = mybir.MatmulPerfMode.DoubleRow


def build(use_bproj=True, use_bfc2=True):
    nc = bacc.Bacc("TRN2", target_bir_lowering=False, debug=False, num_devices=NCORES)

    def inp(name, shape, dt=F32):
        return nc.dram_tensor(name, shape, dt, kind="ExternalInput").ap()

    x_own = inp("x_own", [TOWN, D])
    wq = inp("wq", [D, P], BF16)
    wk = inp("wk", [D, P], BF16)
    wv = inp("wv", [D, P], BF16)
    bq = inp("bq", [P, 1])
    bk = inp("bk", [P, 1])
    bv = inp("bv", [P, 1])
    ln1w = inp("ln1w", [P, 8])
    ln1b = inp("ln1b", [P, 8])
    ln2w = inp("ln2w", [P, 8])
    ln2b = inp("ln2b", [P, 8])
    wproj = inp("wproj", [D, D], BF16)
    bproj = inp("bproj", [1, D], BF16)
    wfc = inp("wfc", [DFF, D] if FC1_DR else [D, DFF],
              FP8 if FC1_DR else BF16)
    bfc = inp("bfc", [P, DFF // P])
    wfc2 = inp("wfc2", [DFF, D], FP8 if FC2_DR else BF16)
    bfc2 = inp("bfc2", [1, D], BF16)
    out_own = nc.dram_tensor("out", [TOWN, D], F32, kind="ExternalOutput").ap()

    groups = [list(range(NCORES))]

    with tile.TileContext(nc) as tc:
        with (
            tc.tile_pool(name="const", bufs=1) as cst,
            tc.tile_pool(name="dram", bufs=1, space="DRAM") as dram,
        ):
            # ---------------- constants ----------------
            ident = cst.tile([P, P], F32)
            make_identity(nc, ident)
            ident_bf = cst.tile([P, P], BF16)
            make_identity(nc, ident_bf)
            ones_b = None
            ones16_b = None
            if use_bproj or use_bfc2:
                ones_f = cst.tile([1, P], F32)
                nc.vector.memset(ones_f[:], 1.0)
                ones_b = cst.tile([1, P], BF16)
                nc.scalar.copy(ones_b[:], ones_f[:])
                if use_bfc2 and FC2_DR:
                    ones16_f = cst.tile([1, P], F32)
                    nc.vector.memset(ones16_f[:], WSC)
                    ones16_b = cst.tile([1, P], BF16)
                    nc.scalar.copy(ones16_b[:], ones16_f[:])
            ln1w_sb = cst.tile([P, 8], F32)
            nc.sync.dma_start(ln1w_sb[:], ln1w)
            ln1b_sb = cst.tile([P, 8], F32)
            nc.sync.dma_start(ln1b_sb[:], ln1b)
            ln2w_sb = cst.tile([P, 8], F32)
            nc.sync.dma_start(ln2w_sb[:], ln2w)
            ln2b_sb = cst.tile([P, 8], F32)
            nc.sync.dma_start(ln2b_sb[:], ln2b)
            bq_sb = cst.tile([P, 1], F32)
            nc.sync.dma_start(bq_sb[:], bq)
            bk_sb = cst.tile([P, 1], F32)
            nc.sync.dma_start(bk_sb[:], bk)
            bv_sb = cst.tile([P, 1], F32)
            nc.sync.dma_start(bv_sb[:], bv)
            bproj_sb = None
            if use_bproj:
                bproj_sb = cst.tile([1, D], BF16)
                nc.sync.dma_start(bproj_sb[:], bproj)
            bfc_sb = cst.tile([P, DFF // P], F32)
            nc.sync.dma_start(bfc_sb[:], bfc)
            bfc2_sb = None
            if use_bfc2:
                bfc2_sb = cst.tile([1, D], BF16)
                nc.sync.dma_start(bfc2_sb[:], bfc2)
            # 0/1 causal masks for the 4 diagonal offsets, replicated for
            # both heads: mask01[s][i, h, j] = 1 if i <= j - 128*s else 0
            mask01 = cst.tile([P, 4, 2, 512], BF16)
            nc.vector.memset(mask01[:], 1.0)
            for s in range(4):
                for h in range(2):
                    nc.gpsimd.affine_select(
                        out=mask01[:, s, h, :],
                        in_=mask01[:, s, h, :],
                        pattern=[[1, 512]],
                        channel_multiplier=-1,
                        base=-128 * s,
                        compare_op=Alu.is_ge,
                        fill=0.0,
                    )

            # DRAM intermediates.
            # hT chunk ci: own tokens [512ci, 512ci+512) -> gathered chunk
            # ci holds batches {2ci, 2ci+1} as [8 strips, ...].
            NCH = 2
            CHT = TOWN // NCH  # own tokens per chunk
            hT_dram = [dram.tile([D, CHT], HDT, name=f"hq{i}") for i in range(NCH)]
            hT_full = [dram.tile([NCORES * D, CHT], HDT, addr_space="Shared",
                                 name=f"hfq{i}") for i in range(NCH)]
            # per-batch y AllToAll: slice r = my 2 heads' y for core r's
            # strip of this batch.
            y_send = [dram.tile([NCORES, 2, HD, STR], BF16, name=f"ys{b}")
                      for b in range(B)]
            y_recv = [dram.tile([NCORES, 2, HD, STR], BF16, name=f"yr{b}")
                      for b in range(B)]

            # =========================================================
            # Phase 1: LN1 over own tokens, 4 chunks -> AllGather each
            # =========================================================
            def layernorm_tile(pool, xt, w_sb, b_sb, ps_pool, dstT, t):
                """LN a [128, D] token tile and write transposed blocks
                (with gamma/beta applied) into dstT[:, dblk, t, :] (bf16)."""
                ssum = pool.tile([P, 1], F32, tag="ssum")
                nc.vector.reduce_sum(ssum[:], xt[:], axis=AX.X)
                mean = pool.tile([P, 1], F32, tag="mean")
                nc.scalar.mul(mean[:], ssum[:], 1.0 / D)
                sq = pool.tile([P, D], F32, tag="sq")
                sumsq = pool.tile([P, 1], F32, tag="sumsq")
                nc.scalar.activation(sq[:], xt[:], Act.Square, accum_out=sumsq[:])
                msq = pool.tile([P, 1], F32, tag="msq")
                nc.vector.tensor_tensor(msq[:], mean[:], mean[:], op=Alu.mult)
                var = pool.tile([P, 1], F32, tag="var")
                nc.vector.tensor_scalar(var[:], sumsq[:], 1.0 / D, EPS, Alu.mult, Alu.add)
                nc.vector.tensor_tensor(var[:], var[:], msq[:], op=Alu.subtract)
                rinv = pool.tile([P, 1], F32, tag="rinv")
                nc.vector.reciprocal(rinv[:], var[:])
                rstd = pool.tile([P, 1], F32, tag="rstd")
                nc.scalar.sqrt(rstd[:], rinv[:])
                hh = pool.tile([P, D], F32, tag="hh")
                nc.vector.tensor_scalar(
                    hh[:], xt[:], mean[:], rstd[:], Alu.subtract, Alu.mult
                )
                for dblk in range(8):
                    pt = ps_pool.tile([P, P], F32, tag="lnt")
                    nc.tensor.transpose(pt[:], hh[:, dblk * P : (dblk + 1) * P], ident[:])
                    nc.scalar.activation(
                        dstT[:, dblk, t, :],
                        pt[:],
                        Act.Identity,
                        bias=b_sb[:, dblk : dblk + 1],
                        scale=w_sb[:, dblk : dblk + 1],
                    )

            with (
                tc.tile_pool(name="ph1", bufs=2) as ph1,
                tc.tile_pool(name="ph1T", bufs=1) as ph1T,
                tc.tile_pool(name="psA", bufs=2, space="PSUM") as psA,
                nc.named_scope("ph1_ln1"),
            ):
                hT_asm = ph1T.tile([P, 8, 8, P], HDT)  # [p, dblk, t, j]
                TPC = 8 // NCH  # token tiles per chunk
                for ci in range(NCH):
                    for t in range(TPC * ci, TPC * ci + TPC):
                        xt = ph1.tile([P, D], F32, tag="xt")
                        nc.sync.dma_start(xt[:], x_own[t * P : (t + 1) * P, :])
                        layernorm_tile(ph1, xt, ln1w_sb, ln1b_sb, psA, hT_asm, t)
                    hTv = hT_dram[ci].rearrange("(dblk p) t -> p dblk t", p=P)
                    for dblk in range(8):
                        nc.sync.dma_start(
                            hTv[:, dblk, :],
                            hT_asm[:, dblk, TPC * ci : TPC * ci + TPC, :],
                        )
                    nc.gpsimd.collective_compute(
                        "AllGather", Alu.bypass, replica_groups=groups,
                        ins=[hT_dram[ci][:]], outs=[hT_full[ci][:]],
                    )

            # weights for ph4/ph5, prefetched during the AllGather wait
            # window (DMA engines are idle there)
            wpre_cm = tc.tile_pool(name="wpre", bufs=1)
            wpre = wpre_cm.__enter__()
            wproj_sb = wpre.tile([P, 8, D], BF16)
            nc.sync.dma_start(
                wproj_sb[:], wproj.rearrange("(ko p) n -> p ko n", p=P)
            )
            if FC2_DR:
                # host-packed (kq p ko) rows: dff = 256*kq + 128*ko + p
                wfc2_sb = wpre.tile([P, DFF // 256, 2, D], FP8)
                nc.sync.dma_start(
                    wfc2_sb[:],
                    wfc2.rearrange("(kq p ko) n -> p kq ko n", p=P, ko=2),
                )
            else:
                wfc2_sb = wpre.tile([P, DFF // P, D], BF16)
                nc.sync.dma_start(
                    wfc2_sb[:], wfc2.rearrange("(kb p) n -> p kb n", p=P)
                )

            # =========================================================
            # Phase 2: q^T, k^T (bf16, both heads stacked on 128
            # partitions) and v-natural (vn, with 64 ones-columns per
            # head) per gathered chunk.  Gathered chunk ci strip rr
            # holds batches {2ci, 2ci+1}: local t in [0,512) -> batch
            # 2ci + t//256, global tok 2048*(2ci+t//256) + 256*rr + t%256
            # =========================================================
            with tc.tile_pool(name="qkv", bufs=1) as qkvp:
                qT = qkvp.tile([P, 16, 512], BF16)
                kT = qkvp.tile([P, 16, 512], BF16)
                # vn[kv_p, g, h, 0:64] = v dims of head h for kv block g;
                # cols 64:128 = 1.0 (softmax-sum columns for the AV mm)
                vn = qkvp.tile([P, 64, 2, P], BF16)
                nc.vector.memset(vn[:, :, :, HD:P], 1.0)
                with (
                    tc.tile_pool(name="wqkv", bufs=1) as wp,
                    tc.tile_pool(name="ph2", bufs=3) as ph2,
                    tc.tile_pool(name="psB", bufs=3, space="PSUM") as psB,
                    tc.tile_pool(name="psBT", bufs=2, space="PSUM") as psBT,
                    nc.named_scope("ph2_qkv"),
                ):
                    wq_sb = wp.tile([P, 8, P], BF16)
                    nc.sync.dma_start(wq_sb[:], wq.rearrange("(ko p) m -> p ko m", p=P))
                    wk_sb = wp.tile([P, 8, P], BF16)
                    nc.sync.dma_start(wk_sb[:], wk.rearrange("(ko p) m -> p ko m", p=P))
                    wv_sb = wp.tile([P, 8, P], BF16)
                    nc.sync.dma_start(wv_sb[:], wv.rearrange("(ko p) m -> p ko m", p=P))
                    hfvs = [hq.rearrange("(r ko p) t -> r p ko t", p=P, ko=8)
                            for hq in hT_full]
                    for ci in range(NCH):
                        hfv = hfvs[ci]
                        for rr in range(8):
                            ht = ph2.tile([P, 8, CHT], HDT, tag="ht")
                            nc.sync.dma_start(ht[:], hfv[rr])
                            # two 256-token halves -> two tile16 slots
                            t16a = 8 * ci + rr // 2
                            co = (rr % 2) * STR
                            sls = [
                                (slice(None), t16a, slice(co, co + STR)),
                                (slice(None), t16a + 4, slice(co, co + STR)),
                            ]
                            for wi, (w_sb, b_sb, dstT) in enumerate(
                                ((wq_sb, bq_sb, qT), (wk_sb, bk_sb, kT),
                                 (wv_sb, bv_sb, None))
                            ):
                                ps = psB.tile([P, CHT], F32, tag="qkvps")
                                for ko in range(8):
                                    nc.tensor.matmul(
                                        ps[:], w_sb[:, ko, :], ht[:, ko, :],
                                        start=(ko == 0), stop=(ko == 7),
                                    )
                                if dstT is not None:
                                    for half in range(2):
                                        nc.scalar.activation(
                                            dstT[sls[half]],
                                            ps[:, half * STR : half * STR + STR],
                                            Act.Identity, bias=b_sb[:],
                                        )
                                else:
                                    # v: bias, transpose to natural, pack
                                    # into vn (dims on cols, split heads)
                                    vtmp = ph2.tile([P, CHT], BF16, tag="vtmp")
                                    nc.scalar.activation(
                                        vtmp[:], ps[:], Act.Identity,
                                        bias=b_sb[:],
                                    )
                                    for jp in range(2):  # pairs of kv blocks
                                        pt = psBT.tile([P, 2, P], BF16, tag="vt")
                                        for j2 in range(2):
                                            j = jp * 2 + j2
                                            nc.tensor.transpose(
                                                pt[:, j2, :],
                                                vtmp[:, j * P : (j + 1) * P],
                                                ident_bf[:],
                                            )
                                        g0 = 32 * ci + 16 * jp + 2 * rr
                                        nc.vector.tensor_copy(
                                            vn[:, g0 : g0 + 2, :, 0:HD],
                                            pt[:].rearrange(
                                                "p j (h c) -> p j h c", h=2),
                                        )

                # =====================================================
                # Phase 3: causal attention, batch-major; both heads
                # concurrently via row-tiled K=64 score matmuls.
                # =====================================================
                with (
                    tc.tile_pool(name="ph3", bufs=4) as ph3,
                    tc.tile_pool(name="ph3s", bufs=3) as ph3s,
                    tc.tile_pool(name="psQK", bufs=2, space="PSUM") as psQK,
                    tc.tile_pool(name="psY", bufs=2, space="PSUM") as psY,
                    nc.named_scope("ph3_attn"),
                ):
                    for b in range(B):
                        for tq in (3, 2, 1, 0):
                            nkv = 4 * (tq + 1)
                            py = psY.tile([P, 2, 512], F32, tag="py")
                            pend = []  # pipelined AV emission
                            for kb in range(nkv):
                                diag = kb >= 4 * tq
                                ps = psQK.tile([P, 2, 512], F32, tag="qk")
                                for h in range(2):
                                    hp = h * HD
                                    nc.tensor.matmul(
                                        ps[:, h, :],
                                        kT[hp : hp + HD, b * 4 + kb // 4,
                                           (kb % 4) * P : (kb % 4 + 1) * P],
                                        qT[hp : hp + HD, b * 4 + tq, :],
                                        start=True, stop=True,
                                    )
                                # diag block s: cols [0,128s) are fully
                                # causal-invalid -> memset 0, skip the exp
                                # there; cols [128s,128(s+1)) get the 0/1
                                # triangle mask; cols >= 128(s+1) are valid.
                                s_off = kb - 4 * tq
                                lo = P * s_off if (s_off > 0 and DIAG_SKIP) else 0
                                ex = ph3.tile([P, 2, 512], BF16, tag="ex")
                                if lo:
                                    nc.vector.memset(ex[:, :, 0:lo], 0.0)
                                nc.scalar.activation(
                                    ex[:, :, lo:], ps[:, :, lo:], Act.Exp,
                                    scale=1.0 / np.sqrt(HD)
                                )
                                if diag:
                                    mw = P * (s_off + 1)
                                    nc.vector.tensor_tensor(
                                        ex[:, :, lo:mw], ex[:, :, lo:mw],
                                        mask01[:, s_off, :, lo:mw],
                                        op=Alu.mult,
                                    )
                                pend.append((kb, ex))
                                if len(pend) > 1:  # one score block ahead
                                    kb0, e0 = pend.pop(0)
                                    for h in range(2):
                                        nc.tensor.matmul(
                                            py[:, h, :],
                                            vn[:, 16 * b + kb0, h, :],
                                            e0[:, h, :],
                                            start=(kb0 == 0), stop=False,
                                        )
                            while pend:
                                kb0, e0 = pend.pop(0)
                                for h in range(2):
                                    nc.tensor.matmul(
                                        py[:, h, :],
                                        vn[:, 16 * b + kb0, h, :],
                                        e0[:, h, :],
                                        start=(kb0 == 0),
                                        stop=(not pend),
                                    )
                            # normalize: rows 64..127 hold the softmax
                            # sums (replicated by the ones columns)
                            for h in range(2):
                                yt = ph3s.tile([HD, 512], BF16, tag="yt")
                                if USE_DIVIDE:
                                    nc.vector.tensor_tensor(
                                        yt[:], py[0:HD, h, :], py[HD:P, h, :],
                                        op=Alu.divide,
                                    )
                                else:
                                    rec = ph3s.tile([HD, 512], F32, tag="rec")
                                    if FAST_RECIP:
                                        # ~18-bit fast reciprocal: sums are
                                        # in [~0.05, ~5e3], far from the
                                        # undefined edges (0/denorm/inf).
                                        # (the custom-DVE op misreads PSUM,
                                        # so stage the sums through SBUF)
                                        ssb = ph3s.tile([HD, 512], F32,
                                                        tag="ssb")
                                        nc.scalar.copy(ssb[:], py[HD:P, h, :])
                                        nc.vector.reciprocal_approx_fast(
                                            rec[:], ssb[:])
                                    else:
                                        nc.vector.reciprocal(
                                            rec[:], py[HD:P, h, :])
                                    nc.vector.tensor_tensor(
                                        yt[:], py[0:HD, h, :], rec[:], op=Alu.mult
                                    )
                                # tq covers strips r = 2tq, 2tq+1
                                ysv = y_send[b].rearrange("r h p t -> h p r t")
                                nc.sync.dma_start(
                                    ysv[h, :, 2 * tq : 2 * tq + 2, :],
                                    yt[:].rearrange("p (r t) -> p r t", r=2),
                                )
                        with nc.named_scope(f"cc_a2a_y{b}"):
                            nc.gpsimd.collective_compute(
                                "AllToAll", Alu.bypass, replica_groups=groups,
                                ins=[y_send[b][:]], outs=[y_recv[b][:]],
                            )

            # =========================================================
            # Phase 4: proj + residual + LN2 (own tokens) -> h2^T (SBUF)
            # own t-tile t = batch t//2, strip off (t%2)*128
            # =========================================================
            with tc.tile_pool(name="keep", bufs=1) as keep:
                h2T = keep.tile([P, 8, 8, P], FP8 if FC1_DR else BF16)  # [p, dblk, t, j]
                x1_sb = keep.tile([P, 8, D], F32)    # [p, t, d]
                with (
                    tc.tile_pool(name="ph4", bufs=2) as ph4,
                    tc.tile_pool(name="psC", bufs=4, space="PSUM") as psC,
                    tc.tile_pool(name="psD", bufs=2, space="PSUM") as psD,
                    nc.named_scope("ph4_proj_ln2"),
                ):
                    # y_recv[b][src, h, p, t]: ydim = 128*src + 64*h + p
                    yrv = [y_recv[b].rearrange("s h p t -> h p s t")
                           for b in range(B)]
                    for t in range(8):
                        yv = yrv[t // 2]
                        off = (t % 2) * P
                        yt_own = ph4.tile([P, 8, P], BF16, tag="ytown")
                        nc.sync.dma_start(
                            yt_own[0:HD, :, :], yv[0][:, :, off : off + P]
                        )
                        nc.sync.dma_start(
                            yt_own[HD:P, :, :], yv[1][:, :, off : off + P]
                        )
                        xt = ph4.tile([P, D], F32, tag="xt4")
                        nc.sync.dma_start(xt[:], x_own[t * P : (t + 1) * P, :])
                        for n in range(2):
                            pp = psC.tile([P, 512], F32, tag="pj")
                            if use_bproj:
                                nc.tensor.matmul(
                                    pp[:], ones_b[:, :P],
                                    bproj_sb[:, n * 512 : (n + 1) * 512],
                                    start=True, stop=False,
                                )
                            for ko in range(8):
                                nc.tensor.matmul(
                                    pp[:], yt_own[:, ko, :],
                                    wproj_sb[:, ko, n * 512 : (n + 1) * 512],
                                    start=(ko == 0 and not use_bproj),
                                    stop=(ko == 7),
                                )
                            nc.vector.tensor_tensor(
                                x1_sb[:, t, n * 512 : (n + 1) * 512], pp[:],
                                xt[:, n * 512 : (n + 1) * 512], op=Alu.add,
                            )
                        layernorm_tile(
                            ph4, x1_sb[:, t, :], ln2w_sb, ln2b_sb, psD, h2T, t
                        )

                # =====================================================
                # Phase 5: MLP (own tokens, 2 groups of 512, bf16)
                # =====================================================
                with (
                    tc.tile_pool(name="mt", bufs=1) as mtp,
                    tc.tile_pool(name="ph5", bufs=3) as ph5,
                    tc.tile_pool(name="psM", bufs=2, space="PSUM") as psM,
                    tc.tile_pool(name="psO", bufs=1, space="PSUM") as psO,
                    nc.named_scope("ph5_mlp"),
                ):
                    if FC1_DR:
                        wfcv = None  # kb-major host pack, sliced directly
                    else:
                        wfcv = wfc.rearrange("(ko p) n -> p ko n", p=P)
                    NKB = DFF // P
                    for g in range(2):
                        mT = mtp.tile([P, NKB, 512], FP8 if FC2_DR else BF16,
                                      tag="mt")
                        for kb in range(NKB):
                            pm = psM.tile([P, 512], F32, tag="pm")
                            if FC1_DR:
                                wt = ph5.tile([P, 4, 2, P], FP8, tag="wfct")
                                nc.sync.dma_start(
                                    wt[:], wfc[kb * P : (kb + 1) * P, :]
                                )
                                for kq in range(4):
                                    nc.tensor.matmul(
                                        pm[:], wt[:, kq, :, :],
                                        h2T[:, 2 * kq : 2 * kq + 2,
                                            4 * g : 4 * g + 4, :],
                                        start=(kq == 0), stop=(kq == 3),
                                        perf_mode=DR,
                                    )
                            else:
                                wt = ph5.tile([P, 8, P], BF16, tag="wfct")
                                nc.sync.dma_start(
                                    wt[:], wfcv[:, :, kb * P : (kb + 1) * P]
                                )
                                for ko in range(8):
                                    nc.tensor.matmul(
                                        pm[:], wt[:, ko, :],
                                        h2T[:, ko, 4 * g : 4 * g + 4, :],
                                        start=(ko == 0), stop=(ko == 7),
                                    )
                            # fp8 weights are prescaled by WSC; the gelu
                            # activation applies scale before the bias add
                            nc.scalar.activation(
                                mT[:, kb, :], pm[:], Act.Gelu,
                                bias=bfc_sb[:, kb : kb + 1],
                                scale=(1.0 / WSC) if FC1_DR else 1.0,
                            )
                        for n2 in range(2):
                            pos = [
                                psO.tile([P, 512], F32, tag=f"po{t2}",
                                         name=f"po_{g}_{n2}_{t2}")
                                for t2 in range(4)
                            ]
                            if use_bfc2:
                                for t2 in range(4):
                                    nc.tensor.matmul(
                                        pos[t2][:],
                                        (ones16_b if FC2_DR else ones_b)[:, :P],
                                        bfc2_sb[:, n2 * 512 : (n2 + 1) * 512],
                                        start=True, stop=False,
                                    )
                            if FC2_DR:
                                for kq in range(DFF // 256):
                                    for t2 in range(4):
                                        nc.tensor.matmul(
                                            pos[t2][:],
                                            mT[:, 2 * kq : 2 * kq + 2,
                                               t2 * P : (t2 + 1) * P],
                                            wfc2_sb[:, kq, :,
                                                    n2 * 512 : (n2 + 1) * 512],
                                            start=(kq == 0 and not use_bfc2),
                                            stop=(kq == DFF // 256 - 1),
                                            perf_mode=DR,
                                        )
                            else:
                                for kb in range(NKB):
                                    for t2 in range(4):
                                        nc.tensor.matmul(
                                            pos[t2][:],
                                            mT[:, kb, t2 * P : (t2 + 1) * P],
                                            wfc2_sb[:, kb,
                                                    n2 * 512 : (n2 + 1) * 512],
                                            start=(kb == 0 and not use_bfc2),
                                            stop=(kb == NKB - 1),
                                        )
                            for t2 in range(4):
                                t = 4 * g + t2
                                ot = ph5.tile([P, 512], F32, tag="ot")
                                if FC2_DR:
                                    nc.vector.scalar_tensor_tensor(
                                        ot[:], pos[t2][:], 1.0 / WSC,
                                        x1_sb[:, t, n2 * 512 : (n2 + 1) * 512],
                                        op0=Alu.mult, op1=Alu.add,
                                    )
                                else:
                                    nc.vector.tensor_tensor(
                                        ot[:], pos[t2][:],
                                        x1_sb[:, t, n2 * 512 : (n2 + 1) * 512],
                                        op=Alu.add,
                                    )
                                nc.sync.dma_start(
                                    out_own[t * P : (t + 1) * P,
                                            n2 * 512 : (n2 + 1) * 512],
                                    ot[:],
                                )
            wpre_cm.__exit__(None, None, None)

    nc.compile()
    return nc


_NC_CACHE = {}


def kernel(x, ln1_w, ln1_b, ln2_w, ln2_b, w_attn, b_attn, w_proj, b_proj,
           w_fc, b_fc, w_fc2, b_fc2):
    f = np.ascontiguousarray
    x = np.asarray(x, np.float32)
    w_attn = np.asarray(w_attn, np.float32)
    b_attn = np.asarray(b_attn, np.float32)
    b_proj = np.asarray(b_proj, np.float32)
    b_fc2 = np.asarray(b_fc2, np.float32)

    key = (bool(np.any(b_proj)), bool(np.any(b_fc2)))
    if key not in _NC_CACHE:
        _NC_CACHE[key] = build(use_bproj=key[0], use_bfc2=key[1])
    nc = _NC_CACHE[key]
    global _NC_LAST
    _NC_LAST = nc

    # striped ownership: core c owns strip c of every batch
    xs = x.reshape(B, NCORES, STR, D)  # [b, strip, 256, d]

    def col(v, c):  # [128, 1] bias slice
        return f(np.asarray(v, np.float32)[c * P : (c + 1) * P].reshape(P, 1))

    def strip(v):  # [1024] -> [128, 8] with [p, a] = v[a*128 + p]
        return f(np.asarray(v, np.float32).reshape(-1, P).T)

    def bf(v):
        return f(np.asarray(v, np.float32).astype(BF))

    def pack_dr(w):
        # [K, N] -> fp8 rows reordered so row (256*kq + 128*ko + p) pairs
        # (p, ko) for DoubleRow; prescaled by WSC to clear e4m3 denormals
        import ml_dtypes as mld
        w = np.asarray(w, np.float32) * WSC
        K, N = w.shape
        w = w.reshape(K // 256, 2, P, N).transpose(0, 2, 1, 3).reshape(K, N)
        return f(w.astype(mld.float8_e4m3))

    def pack_dr_kb(w):
        # fc1 [K, N] -> [N/128 * 128, K/256 * 2 * 128] fp8 laid out
        # [kb, p, kq, ko, n] so the per-kb DMA slice is contiguous
        import ml_dtypes as mld
        w = np.asarray(w, np.float32) * WSC
        K, N = w.shape
        arr = w.reshape(K // 256, 2, P, N // P, P).transpose(3, 2, 0, 1, 4)
        return f(arr.reshape(N, K).astype(mld.float8_e4m3))

    in_maps = []
    for c in range(NCORES):
        in_maps.append({
            "x_own": f(xs[:, c].reshape(TOWN, D)),
            "wq": bf(w_attn[:, P * c : P * (c + 1)]),
            "wk": bf(w_attn[:, D + P * c : D + P * (c + 1)]),
            "wv": bf(w_attn[:, 2 * D + P * c : 2 * D + P * (c + 1)]),
            "bq": col(b_attn, c),
            "bk": col(b_attn[D:], c),
            "bv": col(b_attn[2 * D:], c),
            "ln1w": strip(ln1_w), "ln1b": strip(ln1_b),
            "ln2w": strip(ln2_w), "ln2b": strip(ln2_b),
            "wproj": bf(w_proj),
            "bproj": bf(b_proj.reshape(1, D)),
            "wfc": pack_dr_kb(w_fc) if FC1_DR else bf(w_fc),
            "bfc": strip(b_fc),
            "wfc2": pack_dr(w_fc2) if FC2_DR else bf(w_fc2),
            "bfc2": bf(b_fc2.reshape(1, D)),
        })

    global _last_in_maps
    _last_in_maps = in_maps
    res = run_bass_kernel_spmd(nc, in_maps, core_ids=list(range(NCORES)))
    # reassemble: core c's rows are [b, strip c] pieces
    out = np.empty((B, NCORES, STR, D), np.float32)
    for c in range(NCORES):
        out[:, c] = res.results[c]["out"].reshape(B, STR, D)
    return out.reshape(B, T, D)


_NC_LAST = None
_last_in_maps = None


# revision 29
# speedup vs baseline: 1.4322x; 1.0810x over previous
"""Trainium2 Bass kernel for a GPT-style transformer block.

Reference computation (B=4, T=2048, d=1024, 16 heads, dff=4096, fp32):
    h  = LN1(x);  qkv = h @ w_attn + b_attn
    y  = causal_attention(q, k, v);  x1 = x + y @ w_proj + b_proj
    h2 = LN2(x1); out = x1 + gelu(h2 @ w_fc + b_fc) @ w_fc2 + b_fc2

Sharding over 8 NeuronCores (one trn2 chip), STRIPED token ownership:
  core c owns token strip [2048*b + 256*c, +256) of every batch b (1024
  tokens total).  The striping aligns LN1-AllGather chunks with whole
  batches and lets the per-head attention output redistribute with one
  small AllToAll per batch, pipelined behind the next batch's attention
  (a contiguous split would force a monolithic, exposed AllToAll).

  - ph1: LN1 over own tokens in 2 chunks of 512; each chunk's h^T shard
    (fp8e4m3 - LN bounds the range, and it halves the collective bytes)
    AllGathers as soon as it is ready.
  - ph2: per gathered chunk, q^T/k^T (both heads stacked on the 128
    partitions) and v-natural (vn, with 64 ones-columns per head) for
    this core's 2 heads, via fp8 DoubleRow matmuls (K=256 per pass,
    weights host-prescaled x16 out of the e4m3 denormal range and
    rescaled in the PSUM-eviction activation).
  - ph3: causal attention, batch-major.  Scores for the two heads run
    CONCURRENTLY as row-tiled K=64 matmuls (row groups 0-63 / 64-127 of
    the PE array) into separate PSUM banks - matmul time is free-dim
    cycles only, so this halves score time.  exp on ACT skips the
    fully-masked low columns of diagonal blocks (memset 0 instead);
    the triangle itself is a 0/1 bf16 multiply on DVE.  The softmax
    denominator comes from the ones-columns in vn (AV matmul emits the
    sums on PSUM rows 64..127); normalization uses
    reciprocal_approx_fast (5x faster than DVE reciprocal; the sums are
    staged through SBUF because the custom-DVE op misreads PSUM).
    After each batch one AllToAll (512 KB) redistributes y, hidden
    behind the next batch.
  - ph4: token-parallel proj + residual + LN2 (h2^T written in fp8).
    wproj/wfc2 are prefetched during the ph1 AllGather window.
  - ph5: MLP.  fc1 in bf16 (fp8 here fails the 2e-2 gate - gelu+fc2
    amplify its noise), fc2 in fp8 DoubleRow with the residual add and
    1/16 rescale fused into one DVE scalar_tensor_tensor.

Bias matmuls (ones-row trick) are skipped at build time when the bias
vectors are all zero (they are, for this problem's inputs); nonzero
biases still work through the general path.

Measured: HW exec ~680-720us vs 843us baseline; rel err 1.69e-2
(budget 2e-2; error is dominated by the fp8 paths and is deterministic
for the fixed-seed inputs).
"""

import sys

import numpy as np
import ml_dtypes

sys.path.insert(0, "/opt/trn_rl_repo")

import concourse.bass as bass  # noqa: E402
import concourse.mybir as mybir  # noqa: E402
import concourse.tile as tile  # noqa: E402
from concourse import bacc  # noqa: E402
from concourse.bass_utils import run_bass_kernel_spmd  # noqa: E402
from concourse.masks import make_identity  # noqa: E402

B, T, D, H, HD, DFF = 4, 2048, 1024, 16, 64, 4096
EPS = 1e-5
NCORES = 8
TOK = B * T            # 8192 flattened tokens
TOWN = TOK // NCORES   # 1024 tokens owned per core
STR = 256              # per-batch strip owned per core
P = 128
F32 = mybir.dt.float32
BF16 = mybir.dt.bfloat16
FP8 = mybir.dt.float8e4
Act = mybir.ActivationFunctionType
Alu = mybir.AluOpType
AX = mybir.AxisListType
BF = ml_dtypes.bfloat16

H_FP8 = True        # gather h in fp8e4m3 (halves the AllGather bytes)
FC1_DR = False      # fc1 fp8 fails the 2e-2 gate (gelu+fc2 amplify noise)
QKV_DR = True       # qkv projections in fp8 DoubleRow (h is already fp8)
FC2_DR = True       # fc2 in fp8e4m3 with DoubleRow (K=256 per pass)
USE_DIVIDE = False  # DVE tensor_tensor divide fails neuronxcc codegen
FAST_RECIP = True   # reciprocal_approx_fast for softmax denominators
DIAG_SKIP = True    # skip exp on fully-invalid diag columns
HDT = FP8 if H_FP8 else BF16
WSC = 16.0  # fp8 weight pre-scale (w*0.02 sigma is denormal in e4m3)
DR = mybir.MatmulPerfMode.DoubleRow


def build(use_bproj=True, use_bfc2=True):
    nc = bacc.Bacc("TRN2", target_bir_lowering=False, debug=False, num_devices=NCORES)

    def inp(name, shape, dt=F32):
        return nc.dram_tensor(name, shape, dt, kind="ExternalInput").ap()

    x_own = inp("x_own", [TOWN, D])
    wq = inp("wq", [D, P], FP8 if QKV_DR else BF16)
    wk = inp("wk", [D, P], FP8 if QKV_DR else BF16)
    wv = inp("wv", [D, P], FP8 if QKV_DR else BF16)
    bq = inp("bq", [P, 1])
    bk = inp("bk", [P, 1])
    bv = inp("bv", [P, 1])
    ln1w = inp("ln1w", [P, 8])
    ln1b = inp("ln1b", [P, 8])
    ln2w = inp("ln2w", [P, 8])
    ln2b = inp("ln2b", [P, 8])
    wproj = inp("wproj", [D, D], BF16)
    bproj = inp("bproj", [1, D], BF16)
    wfc = inp("wfc", [DFF, D] if FC1_DR else [D, DFF],
              FP8 if FC1_DR else BF16)
    bfc = inp("bfc", [P, DFF // P])
    wfc2 = inp("wfc2", [DFF, D], FP8 if FC2_DR else BF16)
    bfc2 = inp("bfc2", [1, D], BF16)
    out_own = nc.dram_tensor("out", [TOWN, D], F32, kind="ExternalOutput").ap()

    groups = [list(range(NCORES))]

    with tile.TileContext(nc) as tc:
        with (
            tc.tile_pool(name="const", bufs=1) as cst,
            tc.tile_pool(name="dram", bufs=1, space="DRAM") as dram,
        ):
            # ---------------- constants ----------------
            ident = cst.tile([P, P], F32)
            make_identity(nc, ident)
            ident_bf = cst.tile([P, P], BF16)
            make_identity(nc, ident_bf)
            ones_b = None
            ones16_b = None
            if use_bproj or use_bfc2:
                ones_f = cst.tile([1, P], F32)
                nc.vector.memset(ones_f[:], 1.0)
                ones_b = cst.tile([1, P], BF16)
                nc.scalar.copy(ones_b[:], ones_f[:])
                if use_bfc2 and FC2_DR:
                    ones16_f = cst.tile([1, P], F32)
                    nc.vector.memset(ones16_f[:], WSC)
                    ones16_b = cst.tile([1, P], BF16)
                    nc.scalar.copy(ones16_b[:], ones16_f[:])
            ln1w_sb = cst.tile([P, 8], F32)
            nc.sync.dma_start(ln1w_sb[:], ln1w)
            ln1b_sb = cst.tile([P, 8], F32)
            nc.sync.dma_start(ln1b_sb[:], ln1b)
            ln2w_sb = cst.tile([P, 8], F32)
            nc.sync.dma_start(ln2w_sb[:], ln2w)
            ln2b_sb = cst.tile([P, 8], F32)
            nc.sync.dma_start(ln2b_sb[:], ln2b)
            bq_sb = cst.tile([P, 1], F32)
            nc.sync.dma_start(bq_sb[:], bq)
            bk_sb = cst.tile([P, 1], F32)
            nc.sync.dma_start(bk_sb[:], bk)
            bv_sb = cst.tile([P, 1], F32)
            nc.sync.dma_start(bv_sb[:], bv)
            bproj_sb = None
            if use_bproj:
                bproj_sb = cst.tile([1, D], BF16)
                nc.sync.dma_start(bproj_sb[:], bproj)
            bfc_sb = cst.tile([P, DFF // P], F32)
            nc.sync.dma_start(bfc_sb[:], bfc)
            bfc2_sb = None
            if use_bfc2:
                bfc2_sb = cst.tile([1, D], BF16)
                nc.sync.dma_start(bfc2_sb[:], bfc2)
            # 0/1 causal masks for the 4 diagonal offsets, replicated for
            # both heads: mask01[s][i, h, j] = 1 if i <= j - 128*s else 0
            mask01 = cst.tile([P, 4, 2, 512], BF16)
            nc.vector.memset(mask01[:], 1.0)
            for s in range(4):
                for h in range(2):
                    nc.gpsimd.affine_select(
                        out=mask01[:, s, h, :],
                        in_=mask01[:, s, h, :],
                        pattern=[[1, 512]],
                        channel_multiplier=-1,
                        base=-128 * s,
                        compare_op=Alu.is_ge,
                        fill=0.0,
                    )

            # DRAM intermediates.
            # hT chunk ci: own tokens [512ci, 512ci+512) -> gathered chunk
            # ci holds batches {2ci, 2ci+1} as [8 strips, ...].
            NCH = 2
            CHT = TOWN // NCH  # own tokens per chunk
            hT_dram = [dram.tile([D, CHT], HDT, name=f"hq{i}") for i in range(NCH)]
            hT_full = [dram.tile([NCORES * D, CHT], HDT, addr_space="Shared",
                                 name=f"hfq{i}") for i in range(NCH)]
            # per-batch y AllToAll: slice r = my 2 heads' y for core r's
            # strip of this batch.
            y_send = [dram.tile([NCORES, 2, HD, STR], BF16, name=f"ys{b}")
                      for b in range(B)]
            y_recv = [dram.tile([NCORES, 2, HD, STR], BF16, name=f"yr{b}")
                      for b in range(B)]

            # =========================================================
            # Phase 1: LN1 over own tokens, 4 chunks -> AllGather each
            # =========================================================
            def layernorm_tile(pool, xt, w_sb, b_sb, ps_pool, dstT, t):
                """LN a [128, D] token tile and write transposed blocks
                (with gamma/beta applied) into dstT[:, dblk, t, :] (bf16)."""
                ssum = pool.tile([P, 1], F32, tag="ssum")
                nc.vector.reduce_sum(ssum[:], xt[:], axis=AX.X)
                mean = pool.tile([P, 1], F32, tag="mean")
                nc.scalar.mul(mean[:], ssum[:], 1.0 / D)
                sq = pool.tile([P, D], F32, tag="sq")
                sumsq = pool.tile([P, 1], F32, tag="sumsq")
                nc.scalar.activation(sq[:], xt[:], Act.Square, accum_out=sumsq[:])
                msq = pool.tile([P, 1], F32, tag="msq")
                nc.vector.tensor_tensor(msq[:], mean[:], mean[:], op=Alu.mult)
                var = pool.tile([P, 1], F32, tag="var")
                nc.vector.tensor_scalar(var[:], sumsq[:], 1.0 / D, EPS, Alu.mult, Alu.add)
                nc.vector.tensor_tensor(var[:], var[:], msq[:], op=Alu.subtract)
                rinv = pool.tile([P, 1], F32, tag="rinv")
                nc.vector.reciprocal(rinv[:], var[:])
                rstd = pool.tile([P, 1], F32, tag="rstd")
                nc.scalar.sqrt(rstd[:], rinv[:])
                hh = pool.tile([P, D], F32, tag="hh")
                nc.vector.tensor_scalar(
                    hh[:], xt[:], mean[:], rstd[:], Alu.subtract, Alu.mult
                )
                for dblk in range(8):
                    pt = ps_pool.tile([P, P], F32, tag="lnt")
                    nc.tensor.transpose(pt[:], hh[:, dblk * P : (dblk + 1) * P], ident[:])
                    nc.scalar.activation(
                        dstT[:, dblk, t, :],
                        pt[:],
                        Act.Identity,
                        bias=b_sb[:, dblk : dblk + 1],
                        scale=w_sb[:, dblk : dblk + 1],
                    )

            with (
                tc.tile_pool(name="ph1", bufs=2) as ph1,
                tc.tile_pool(name="ph1T", bufs=1) as ph1T,
                tc.tile_pool(name="psA", bufs=2, space="PSUM") as psA,
                nc.named_scope("ph1_ln1"),
            ):
                hT_asm = ph1T.tile([P, 8, 8, P], HDT)  # [p, dblk, t, j]
                TPC = 8 // NCH  # token tiles per chunk
                for ci in range(NCH):
                    for t in range(TPC * ci, TPC * ci + TPC):
                        xt = ph1.tile([P, D], F32, tag="xt")
                        nc.sync.dma_start(xt[:], x_own[t * P : (t + 1) * P, :])
                        layernorm_tile(ph1, xt, ln1w_sb, ln1b_sb, psA, hT_asm, t)
                    hTv = hT_dram[ci].rearrange("(dblk p) t -> p dblk t", p=P)
                    for dblk in range(8):
                        nc.sync.dma_start(
                            hTv[:, dblk, :],
                            hT_asm[:, dblk, TPC * ci : TPC * ci + TPC, :],
                        )
                    nc.gpsimd.collective_compute(
                        "AllGather", Alu.bypass, replica_groups=groups,
                        ins=[hT_dram[ci][:]], outs=[hT_full[ci][:]],
                    )

            # weights for ph4/ph5, prefetched during the AllGather wait
            # window (DMA engines are idle there)
            wpre_cm = tc.tile_pool(name="wpre", bufs=1)
            wpre = wpre_cm.__enter__()
            wproj_sb = wpre.tile([P, 8, D], BF16)
            nc.sync.dma_start(
                wproj_sb[:], wproj.rearrange("(ko p) n -> p ko n", p=P)
            )
            if FC2_DR:
                # host-packed (kq p ko) rows: dff = 256*kq + 128*ko + p
                wfc2_sb = wpre.tile([P, DFF // 256, 2, D], FP8)
                nc.sync.dma_start(
                    wfc2_sb[:],
                    wfc2.rearrange("(kq p ko) n -> p kq ko n", p=P, ko=2),
                )
            else:
                wfc2_sb = wpre.tile([P, DFF // P, D], BF16)
                nc.sync.dma_start(
                    wfc2_sb[:], wfc2.rearrange("(kb p) n -> p kb n", p=P)
                )

            # =========================================================
            # Phase 2: q^T, k^T (bf16, both heads stacked on 128
            # partitions) and v-natural (vn, with 64 ones-columns per
            # head) per gathered chunk.  Gathered chunk ci strip rr
            # holds batches {2ci, 2ci+1}: local t in [0,512) -> batch
            # 2ci + t//256, global tok 2048*(2ci+t//256) + 256*rr + t%256
            # =========================================================
            with tc.tile_pool(name="qkv", bufs=1) as qkvp:
                qT = qkvp.tile([P, 16, 512], BF16)
                kT = qkvp.tile([P, 16, 512], BF16)
                # vn[kv_p, g, h, 0:64] = v dims of head h for kv block g;
                # cols 64:128 = 1.0 (softmax-sum columns for the AV mm)
                vn = qkvp.tile([P, 64, 2, P], BF16)
                nc.vector.memset(vn[:, :, :, HD:P], 1.0)
                with (
                    tc.tile_pool(name="wqkv", bufs=1) as wp,
                    tc.tile_pool(name="ph2", bufs=3) as ph2,
                    tc.tile_pool(name="psB", bufs=3, space="PSUM") as psB,
                    tc.tile_pool(name="psBT", bufs=2, space="PSUM") as psBT,
                    nc.named_scope("ph2_qkv"),
                ):
                    if QKV_DR:
                        wsbs = []
                        for wnm, wap in (("q", wq), ("k", wk), ("v", wv)):
                            wsb = wp.tile([P, 4, 2, P], FP8, name=f"w{wnm}dr")
                            nc.sync.dma_start(
                                wsb[:],
                                wap.rearrange("(kq p ko) m -> p kq ko m",
                                              p=P, ko=2))
                            wsbs.append(wsb)
                        wq_sb, wk_sb, wv_sb = wsbs
                    else:
                        wq_sb = wp.tile([P, 8, P], BF16)
                        nc.sync.dma_start(wq_sb[:], wq.rearrange("(ko p) m -> p ko m", p=P))
                        wk_sb = wp.tile([P, 8, P], BF16)
                        nc.sync.dma_start(wk_sb[:], wk.rearrange("(ko p) m -> p ko m", p=P))
                        wv_sb = wp.tile([P, 8, P], BF16)
                        nc.sync.dma_start(wv_sb[:], wv.rearrange("(ko p) m -> p ko m", p=P))
                    hfvs = [hq.rearrange("(r ko p) t -> r p ko t", p=P, ko=8)
                            for hq in hT_full]
                    for ci in range(NCH):
                        hfv = hfvs[ci]
                        for rr in range(8):
                            ht = ph2.tile([P, 8, CHT], HDT, tag="ht")
                            nc.sync.dma_start(ht[:], hfv[rr])
                            # two 256-token halves -> two tile16 slots
                            t16a = 8 * ci + rr // 2
                            co = (rr % 2) * STR
                            sls = [
                                (slice(None), t16a, slice(co, co + STR)),
                                (slice(None), t16a + 4, slice(co, co + STR)),
                            ]
                            for wi, (w_sb, b_sb, dstT) in enumerate(
                                ((wq_sb, bq_sb, qT), (wk_sb, bk_sb, kT),
                                 (wv_sb, bv_sb, None))
                            ):
                                ps = psB.tile([P, CHT], F32, tag="qkvps")
                                if QKV_DR:
                                    for kq in range(4):
                                        nc.tensor.matmul(
                                            ps[:], w_sb[:, kq, :, :],
                                            ht[:, 2 * kq : 2 * kq + 2, :],
                                            start=(kq == 0), stop=(kq == 3),
                                            perf_mode=DR,
                                        )
                                else:
                                    for ko in range(8):
                                        nc.tensor.matmul(
                                            ps[:], w_sb[:, ko, :], ht[:, ko, :],
                                            start=(ko == 0), stop=(ko == 7),
                                        )
                                if dstT is not None:
                                    for half in range(2):
                                        nc.scalar.activation(
                                            dstT[sls[half]],
                                            ps[:, half * STR : half * STR + STR],
                                            Act.Identity, bias=b_sb[:],
                                            scale=(1.0 / WSC) if QKV_DR else 1.0,
                                        )
                                else:
                                    # v: bias, transpose to natural, pack
                                    # into vn (dims on cols, split heads)
                                    vtmp = ph2.tile([P, CHT], BF16, tag="vtmp")
                                    nc.scalar.activation(
                                        vtmp[:], ps[:], Act.Identity,
                                        bias=b_sb[:],
                                        scale=(1.0 / WSC) if QKV_DR else 1.0,
                                    )
                                    for jp in range(2):  # pairs of kv blocks
                                        pt = psBT.tile([P, 2, P], BF16, tag="vt")
                                        for j2 in range(2):
                                            j = jp * 2 + j2
                                            nc.tensor.transpose(
                                                pt[:, j2, :],
                                                vtmp[:, j * P : (j + 1) * P],
                                                ident_bf[:],
                                            )
                                        g0 = 32 * ci + 16 * jp + 2 * rr
                                        nc.vector.tensor_copy(
                                            vn[:, g0 : g0 + 2, :, 0:HD],
                                            pt[:].rearrange(
                                                "p j (h c) -> p j h c", h=2),
                                        )

                # =====================================================
                # Phase 3: causal attention, batch-major; both heads
                # concurrently via row-tiled K=64 score matmuls.
                # =====================================================
                with (
                    tc.tile_pool(name="ph3", bufs=4) as ph3,
                    tc.tile_pool(name="ph3s", bufs=3) as ph3s,
                    tc.tile_pool(name="psQK", bufs=2, space="PSUM") as psQK,
                    tc.tile_pool(name="psY", bufs=2, space="PSUM") as psY,
                    nc.named_scope("ph3_attn"),
                ):
                    for b in range(B):
                        for tq in (3, 2, 1, 0):
                            nkv = 4 * (tq + 1)
                            py = psY.tile([P, 2, 512], F32, tag="py")
                            pend = []  # pipelined AV emission
                            for kb in range(nkv):
                                diag = kb >= 4 * tq
                                ps = psQK.tile([P, 2, 512], F32, tag="qk")
                                for h in range(2):
                                    hp = h * HD
                                    nc.tensor.matmul(
                                        ps[:, h, :],
                                        kT[hp : hp + HD, b * 4 + kb // 4,
                                           (kb % 4) * P : (kb % 4 + 1) * P],
                                        qT[hp : hp + HD, b * 4 + tq, :],
                                        start=True, stop=True,
                                    )
                                # diag block s: cols [0,128s) are fully
                                # causal-invalid -> memset 0, skip the exp
                                # there; cols [128s,128(s+1)) get the 0/1
                                # triangle mask; cols >= 128(s+1) are valid.
                                s_off = kb - 4 * tq
                                lo = P * s_off if (s_off > 0 and DIAG_SKIP) else 0
                                ex = ph3.tile([P, 2, 512], BF16, tag="ex")
                                if lo:
                                    nc.vector.memset(ex[:, :, 0:lo], 0.0)
                                nc.scalar.activation(
                                    ex[:, :, lo:], ps[:, :, lo:], Act.Exp,
                                    scale=1.0 / np.sqrt(HD)
                                )
                                if diag:
                                    mw = P * (s_off + 1)
                                    nc.vector.tensor_tensor(
                                        ex[:, :, lo:mw], ex[:, :, lo:mw],
                                        mask01[:, s_off, :, lo:mw],
                                        op=Alu.mult,
                                    )
                                pend.append((kb, ex))
                                if len(pend) > 1:  # one score block ahead
                                    kb0, e0 = pend.pop(0)
                                    for h in range(2):
                                        nc.tensor.matmul(
                                            py[:, h, :],
                                            vn[:, 16 * b + kb0, h, :],
                                            e0[:, h, :],
                                            start=(kb0 == 0), stop=False,
                                        )
                            while pend:
                                kb0, e0 = pend.pop(0)
                                for h in range(2):
                                    nc.tensor.matmul(
                                        py[:, h, :],
                                        vn[:, 16 * b + kb0, h, :],
                                        e0[:, h, :],
                                        start=(kb0 == 0),
                                        stop=(not pend),
                                    )
                            # normalize: rows 64..127 hold the softmax
                            # sums (replicated by the ones columns)
                            for h in range(2):
                                yt = ph3s.tile([HD, 512], BF16, tag="yt")
                                if USE_DIVIDE:
                                    nc.vector.tensor_tensor(
                                        yt[:], py[0:HD, h, :], py[HD:P, h, :],
                                        op=Alu.divide,
                                    )
                                else:
                                    rec = ph3s.tile([HD, 512], F32, tag="rec")
                                    if FAST_RECIP:
                                        # ~18-bit fast reciprocal: sums are
                                        # in [~0.05, ~5e3], far from the
                                        # undefined edges (0/denorm/inf).
                                        # (the custom-DVE op misreads PSUM,
                                        # so stage the sums through SBUF)
                                        ssb = ph3s.tile([HD, 512], F32,
                                                        tag="ssb")
                                        nc.scalar.copy(ssb[:], py[HD:P, h, :])
                                        nc.vector.reciprocal_approx_fast(
                                            rec[:], ssb[:])
                                    else:
                                        nc.vector.reciprocal(
                                            rec[:], py[HD:P, h, :])
                                    nc.vector.tensor_tensor(
                                        yt[:], py[0:HD, h, :], rec[:], op=Alu.mult
                                    )
                                # tq covers strips r = 2tq, 2tq+1
                                ysv = y_send[b].rearrange("r h p t -> h p r t")
                                nc.sync.dma_start(
                                    ysv[h, :, 2 * tq : 2 * tq + 2, :],
                                    yt[:].rearrange("p (r t) -> p r t", r=2),
                                )
                        with nc.named_scope(f"cc_a2a_y{b}"):
                            nc.gpsimd.collective_compute(
                                "AllToAll", Alu.bypass, replica_groups=groups,
                                ins=[y_send[b][:]], outs=[y_recv[b][:]],
                            )

            # =========================================================
            # Phase 4: proj + residual + LN2 (own tokens) -> h2^T (SBUF)
            # own t-tile t = batch t//2, strip off (t%2)*128
            # =========================================================
            with tc.tile_pool(name="keep", bufs=1) as keep:
                h2T = keep.tile([P, 8, 8, P], FP8 if FC1_DR else BF16)  # [p, dblk, t, j]
                x1_sb = keep.tile([P, 8, D], F32)    # [p, t, d]
                with (
                    tc.tile_pool(name="ph4", bufs=2) as ph4,
                    tc.tile_pool(name="psC", bufs=4, space="PSUM") as psC,
                    tc.tile_pool(name="psD", bufs=2, space="PSUM") as psD,
                    nc.named_scope("ph4_proj_ln2"),
                ):
                    # y_recv[b][src, h, p, t]: ydim = 128*src + 64*h + p
                    yrv = [y_recv[b].rearrange("s h p t -> h p s t")
                           for b in range(B)]
                    for t in range(8):
                        yv = yrv[t // 2]
                        off = (t % 2) * P
                        yt_own = ph4.tile([P, 8, P], BF16, tag="ytown")
                        nc.sync.dma_start(
                            yt_own[0:HD, :, :], yv[0][:, :, off : off + P]
                        )
                        nc.sync.dma_start(
                            yt_own[HD:P, :, :], yv[1][:, :, off : off + P]
                        )
                        xt = ph4.tile([P, D], F32, tag="xt4")
                        nc.sync.dma_start(xt[:], x_own[t * P : (t + 1) * P, :])
                        for n in range(2):
                            pp = psC.tile([P, 512], F32, tag="pj")
                            if use_bproj:
                                nc.tensor.matmul(
                                    pp[:], ones_b[:, :P],
                                    bproj_sb[:, n * 512 : (n + 1) * 512],
                                    start=True, stop=False,
                                )
                            for ko in range(8):
                                nc.tensor.matmul(
                                    pp[:], yt_own[:, ko, :],
                                    wproj_sb[:, ko, n * 512 : (n + 1) * 512],
                                    start=(ko == 0 and not use_bproj),
                                    stop=(ko == 7),
                                )
                            nc.vector.tensor_tensor(
                                x1_sb[:, t, n * 512 : (n + 1) * 512], pp[:],
                                xt[:, n * 512 : (n + 1) * 512], op=Alu.add,
                            )
                        layernorm_tile(
                            ph4, x1_sb[:, t, :], ln2w_sb, ln2b_sb, psD, h2T, t
                        )

                # =====================================================
                # Phase 5: MLP (own tokens, 2 groups of 512, bf16)
                # =====================================================
                with (
                    tc.tile_pool(name="mt", bufs=1) as mtp,
                    tc.tile_pool(name="ph5", bufs=3) as ph5,
                    tc.tile_pool(name="psM", bufs=2, space="PSUM") as psM,
                    tc.tile_pool(name="psO", bufs=1, space="PSUM") as psO,
                    nc.named_scope("ph5_mlp"),
                ):
                    if FC1_DR:
                        wfcv = None  # kb-major host pack, sliced directly
                    else:
                        wfcv = wfc.rearrange("(ko p) n -> p ko n", p=P)
                    NKB = DFF // P
                    for g in range(2):
                        mT = mtp.tile([P, NKB, 512], FP8 if FC2_DR else BF16,
                                      tag="mt")
                        for kb in range(NKB):
                            pm = psM.tile([P, 512], F32, tag="pm")
                            if FC1_DR:
                                wt = ph5.tile([P, 4, 2, P], FP8, tag="wfct")
                                nc.sync.dma_start(
                                    wt[:], wfc[kb * P : (kb + 1) * P, :]
                                )
                                for kq in range(4):
                                    nc.tensor.matmul(
                                        pm[:], wt[:, kq, :, :],
                                        h2T[:, 2 * kq : 2 * kq + 2,
                                            4 * g : 4 * g + 4, :],
                                        start=(kq == 0), stop=(kq == 3),
                                        perf_mode=DR,
                                    )
                            else:
                                wt = ph5.tile([P, 8, P], BF16, tag="wfct")
                                nc.sync.dma_start(
                                    wt[:], wfcv[:, :, kb * P : (kb + 1) * P]
                                )
                                for ko in range(8):
                                    nc.tensor.matmul(
                                        pm[:], wt[:, ko, :],
                                        h2T[:, ko, 4 * g : 4 * g + 4, :],
                                        start=(ko == 0), stop=(ko == 7),
                                    )
                            # fp8 weights are prescaled by WSC; the gelu
                            # activation applies scale before the bias add
                            nc.scalar.activation(
                                mT[:, kb, :], pm[:], Act.Gelu,
                                bias=bfc_sb[:, kb : kb + 1],
                                scale=(1.0 / WSC) if FC1_DR else 1.0,
                            )
                        for n2 in range(2):
                            pos = [
                                psO.tile([P, 512], F32, tag=f"po{t2}",
                                         name=f"po_{g}_{n2}_{t2}")
                                for t2 in range(4)
                            ]
                            if use_bfc2:
                                for t2 in range(4):
                                    nc.tensor.matmul(
                                        pos[t2][:],
                                        (ones16_b if FC2_DR else ones_b)[:, :P],
                                        bfc2_sb[:, n2 * 512 : (n2 + 1) * 512],
                                        start=True, stop=False,
                                    )
                            if FC2_DR:
                                for kq in range(DFF // 256):
                                    for t2 in range(4):
                                        nc.tensor.matmul(
                                            pos[t2][:],
                                            mT[:, 2 * kq : 2 * kq + 2,
                                               t2 * P : (t2 + 1) * P],
                                            wfc2_sb[:, kq, :,
                                                    n2 * 512 : (n2 + 1) * 512],
                                            start=(kq == 0 and not use_bfc2),
                                            stop=(kq == DFF // 256 - 1),
                                            perf_mode=DR,
                                        )
                            else:
                                for kb in range(NKB):
                                    for t2 in range(4):
                                        nc.tensor.matmul(
                                            pos[t2][:],
                                            mT[:, kb, t2 * P : (t2 + 1) * P],
                                            wfc2_sb[:, kb,
                                                    n2 * 512 : (n2 + 1) * 512],
                                            start=(kb == 0 and not use_bfc2),
                                            stop=(kb == NKB - 1),
                                        )
                            for t2 in range(4):
                                t = 4 * g + t2
                                ot = ph5.tile([P, 512], F32, tag="ot")
                                if FC2_DR:
                                    nc.vector.scalar_tensor_tensor(
                                        ot[:], pos[t2][:], 1.0 / WSC,
                                        x1_sb[:, t, n2 * 512 : (n2 + 1) * 512],
                                        op0=Alu.mult, op1=Alu.add,
                                    )
                                else:
                                    nc.vector.tensor_tensor(
                                        ot[:], pos[t2][:],
                                        x1_sb[:, t, n2 * 512 : (n2 + 1) * 512],
                                        op=Alu.add,
                                    )
                                nc.sync.dma_start(
                                    out_own[t * P : (t + 1) * P,
                                            n2 * 512 : (n2 + 1) * 512],
                                    ot[:],
                                )
            wpre_cm.__exit__(None, None, None)

    nc.compile()
    return nc


_NC_CACHE = {}


def kernel(x, ln1_w, ln1_b, ln2_w, ln2_b, w_attn, b_attn, w_proj, b_proj,
           w_fc, b_fc, w_fc2, b_fc2):
    f = np.ascontiguousarray
    x = np.asarray(x, np.float32)
    w_attn = np.asarray(w_attn, np.float32)
    b_attn = np.asarray(b_attn, np.float32)
    b_proj = np.asarray(b_proj, np.float32)
    b_fc2 = np.asarray(b_fc2, np.float32)

    key = (bool(np.any(b_proj)), bool(np.any(b_fc2)))
    if key not in _NC_CACHE:
        _NC_CACHE[key] = build(use_bproj=key[0], use_bfc2=key[1])
    nc = _NC_CACHE[key]
    global _NC_LAST
    _NC_LAST = nc

    # striped ownership: core c owns strip c of every batch
    xs = x.reshape(B, NCORES, STR, D)  # [b, strip, 256, d]

    def col(v, c):  # [128, 1] bias slice
        return f(np.asarray(v, np.float32)[c * P : (c + 1) * P].reshape(P, 1))

    def strip(v):  # [1024] -> [128, 8] with [p, a] = v[a*128 + p]
        return f(np.asarray(v, np.float32).reshape(-1, P).T)

    def bf(v):
        return f(np.asarray(v, np.float32).astype(BF))

    def pack_dr(w):
        # [K, N] -> fp8 rows reordered so row (256*kq + 128*ko + p) pairs
        # (p, ko) for DoubleRow; prescaled by WSC to clear e4m3 denormals
        import ml_dtypes as mld
        w = np.asarray(w, np.float32) * WSC
        K, N = w.shape
        w = w.reshape(K // 256, 2, P, N).transpose(0, 2, 1, 3).reshape(K, N)
        return f(w.astype(mld.float8_e4m3))

    def pack_dr_kb(w):
        # fc1 [K, N] -> [N/128 * 128, K/256 * 2 * 128] fp8 laid out
        # [kb, p, kq, ko, n] so the per-kb DMA slice is contiguous
        import ml_dtypes as mld
        w = np.asarray(w, np.float32) * WSC
        K, N = w.shape
        arr = w.reshape(K // 256, 2, P, N // P, P).transpose(3, 2, 0, 1, 4)
        return f(arr.reshape(N, K).astype(mld.float8_e4m3))

    wpk = pack_dr if QKV_DR else bf

    in_maps = []
    for c in range(NCORES):
        in_maps.append({
            "x_own": f(xs[:, c].reshape(TOWN, D)),
            "wq": wpk(w_attn[:, P * c : P * (c + 1)]),
            "wk": wpk(w_attn[:, D + P * c : D + P * (c + 1)]),
            "wv": wpk(w_attn[:, 2 * D + P * c : 2 * D + P * (c + 1)]),
            "bq": col(b_attn, c),
            "bk": col(b_attn[D:], c),
            "bv": col(b_attn[2 * D:], c),
            "ln1w": strip(ln1_w), "ln1b": strip(ln1_b),
            "ln2w": strip(ln2_w), "ln2b": strip(ln2_b),
            "wproj": bf(w_proj),
            "bproj": bf(b_proj.reshape(1, D)),
            "wfc": pack_dr_kb(w_fc) if FC1_DR else bf(w_fc),
            "bfc": strip(b_fc),
            "wfc2": pack_dr(w_fc2) if FC2_DR else bf(w_fc2),
            "bfc2": bf(b_fc2.reshape(1, D)),
        })

    global _last_in_maps
    _last_in_maps = in_maps
    res = run_bass_kernel_spmd(nc, in_maps, core_ids=list(range(NCORES)))
    # reassemble: core c's rows are [b, strip c] pieces
    out = np.empty((B, NCORES, STR, D), np.float32)
    for c in range(NCORES):
        out[:, c] = res.results[c]["out"].reshape(B, STR, D)
    return out.reshape(B, T, D)


_NC_LAST = None
_last_in_maps = None
